# revision 31
# baseline (speedup 1.0000x reference)
# Trainium2 Bass kernel for nn_EquShiftQ2DF3P40 (group-equivariant CNN + dynamic filter).
#
# Sharding: batch 256 -> 32 samples/core on 8 cores, ALL weights replicated
# (collectives measured far slower than the redundant HBM traffic here).
#
# Optimizations over the original baseline (352us -> ~117us marginal):
# - Wes1 streams as fp8-e3m4 (x256 host scale, folded into bes1t/Wes2):
#   16.7MB instead of 33.5MB on the dominant weight stream. The es1 matmuls
#   are mixed-dtype: bf16 stationary obsT x fp8e3 moving weight chunks
#   (HW-validated exact; final rel err ~1.46e-2 vs the 2e-2 gate).
# - in-hand FC (6400->512) runs as 25 fp8-e4m3 DoubleRow matmuls (K=256
#   per MM); wifp2 streams fp8 (3.3MB). Scales (h x16 via wi2/bi2, wif x64)
#   fold into bifrep/wdf host-side - zero extra device ops.
# - conv1/ihc1 run 4 samples concurrently per piece via 2 row groups x
#   2 col groups (tile_position quads) - PE-array tiling overlaps the
#   K=9 matmuls on hardware (not visible in the cost-model sim).
# - c2's dy2 (K=64) matmuls of an even/odd sample pair are emitted
#   interleaved on disjoint row halves; ihc2 batches two sample-pairs per
#   matmul (N=200) with E/O col groups interleaved.
# - conv weights are pre-swizzled on host to partition-major so their loads
#   are contiguous descriptors; per-chunk dy-stack shift copies overlap the
#   conv loops on the scalar HWDGE ring.
# - the es tail is SPLIT INTO STAGES interleaved into the c2 loop (fold
#   chain: DVE evictions -> partition-shift DMA -> add; then transposes +
#   es2 matmuls), and the dynamic-filter matmuls run mid-c3, so the
#   post-conv critical path is just c4/c5 + the DVE tail.
# - NOTE (measured): merging c2 into the c1 chunk loop, or hoisting the
#   whole es tail as one block before c3, both REGRESSED (+24-40us) -
#   phase-level emission order is fragile because PE executes its queue
#   in order and serial non-PE chains stall it.
import numpy as np
import ml_dtypes

import concourse.bacc as bacc
import concourse.mybir as mybir
from concourse.bass_utils import run_bass_kernel_spmd
from concourse import tile
import bass_rust

f32 = mybir.dt.float32
bf16 = mybir.dt.bfloat16
fp8e3 = mybir.dt.float8e3
fp8e4 = mybir.dt.float8e4
AF = mybir.ActivationFunctionType
ALU = mybir.AluOpType
DR = mybir.MatmulPerfMode.DoubleRow
bf = ml_dtypes.bfloat16
e3m4 = ml_dtypes.float8_e3m4
e4m3 = ml_dtypes.float8_e4m3

NCORES = 8
BC = 32  # samples per core
ES1_GROUPS = 2  # PE column-tile groups for es1 (cols {0,64}, own psum banks)
WES1_SCALE = 256.0   # Wes1 fp8 scale; folded into bes1t (x) and Wes2 (/)
H_SCALE = 16.0       # in-hand conv2 output scale (wi2*, bi2c x16)
WIF_SCALE = 64.0     # Wif fp8 scale; H_SCALE*WIF_SCALE folds into bifrep/wdf
ES1_CHUNKS = 16      # 8 kt per chunk, 1MB fp8 each


# ---------------------------------------------------------------- host prep
def _rot(x, g):
    return np.rot90(x, k=g, axes=(-2, -1))


def _sym(k):
    return 0.5 * (k + _rot(k, 2))


def _expand_tq(kappa):
    kappa = _sym(kappa)
    Co, Ci, kh, kw = kappa.shape
    W = np.stack([_rot(kappa, g) for g in range(2)], axis=1)
    return W.reshape(Co * 2, Ci, kh, kw)


def _expand_qq(kappa):
    kappa = _sym(kappa)
    Co, Ci, F, kh, kw = kappa.shape
    W = np.stack([_rot(np.roll(kappa, g, axis=2), g) for g in range(F)], axis=1)
    return W.reshape(Co * F, Ci * F, kh, kw)


def _pad_flat(x):
    # (B,1,40,40) -> (B, 42*42 + 96) zero-padded flat images
    B = x.shape[0]
    p = np.zeros((B, 42, 42), np.float32)
    p[:, 1:41, 1:41] = x[:, 0]
    out = np.zeros((B, 42 * 42 + 96), np.float32)
    out[:, : 42 * 42] = p.reshape(B, -1)
    return out


def host_prep(inputs):
    """Returns per-core maps of numpy arrays keyed by dram names."""
    obs = np.asarray(inputs["obs_encoding"], np.float32)
    patch = np.asarray(inputs["patch"], np.float32)
    B = obs.shape[0]

    W1e = _expand_tq(np.asarray(inputs["k1"], np.float32))        # (64,1,3,3)
    W2e = _expand_qq(np.asarray(inputs["k2"], np.float32))        # (128,64,3,3)
    W3e = _expand_qq(np.asarray(inputs["k3"], np.float32))        # (256,128,3,3)
    W4e = _expand_qq(np.asarray(inputs["k4"], np.float32))        # (64,256,3,3)
    W5e = _expand_qq(np.asarray(inputs["k5"], np.float32))        # (32,64,3,3)

    sh = {}
    # conv1 / ihc1 im2col weights replicated at row-halves {0,64}
    w1r = np.zeros((128, 64), np.float32)
    wi1r = np.zeros((128, 32), np.float32)
    w19 = W1e[:, 0].reshape(64, 9).T  # (9, 64)
    wi19 = np.asarray(inputs["Wi1"], np.float32)[:, 0].reshape(32, 9).T
    for h in range(2):
        w1r[64 * h:64 * h + 9] = w19
        wi1r[64 * h:64 * h + 9] = wi19
    sh["w1r"] = w1r.astype(bf)
    sh["wi1r"] = wi1r.astype(bf)
    # conv2 dy-stacked: w2a (3,128,128) rows=(dy0 ci | dy1 ci); w2b (3,64,128) dy2
    w2a = np.zeros((3, 128, 128), np.float32)
    w2b = np.zeros((3, 64, 128), np.float32)
    for dx in range(3):
        w2a[dx, 0:64] = W2e[:, :, 0, dx].T
        w2a[dx, 64:128] = W2e[:, :, 1, dx].T
        w2b[dx] = W2e[:, :, 2, dx].T
    # partition-major swizzles for contiguous weight loads
    sh["w2a"] = np.ascontiguousarray(w2a.transpose(1, 0, 2)).astype(bf)       # (128,3,128)
    sh["w2ao"] = np.ascontiguousarray(
        w2a[:, list(range(64, 128)) + list(range(64))].transpose(1, 0, 2)).astype(bf)
    w2b2 = np.zeros((128, 3, 128), np.float32)   # both row halves hold dy2
    w2b2[64:128] = w2b.transpose(1, 0, 2)
    w2b2[0:64] = w2b.transpose(1, 0, 2)
    sh["w2b2"] = w2b2.astype(bf)
    # conv3: [p][(mt dy dx)][o]
    w3 = np.zeros((2, 3, 3, 128, 128), np.float32)
    for mt in range(2):
        for dy in range(3):
            for dx in range(3):
                w3[mt, dy, dx] = W3e[mt * 128:(mt + 1) * 128, :, dy, dx].T
    sh["w3"] = np.ascontiguousarray(w3.transpose(3, 0, 1, 2, 4)).reshape(128, 18, 128).astype(bf)
    # conv4: [p][(kt dy dx)][o]
    w4 = np.zeros((2, 3, 3, 128, 64), np.float32)
    for kt in range(2):
        for dy in range(3):
            for dx in range(3):
                w4[kt, dy, dx] = W4e[:, kt * 128:(kt + 1) * 128, dy, dx].T
    sh["w4"] = np.ascontiguousarray(w4.transpose(3, 0, 1, 2, 4)).reshape(128, 18, 64).astype(bf)
    # conv5: [p][q][o]
    w5 = np.zeros((9, 64, 32), np.float32)
    for dy in range(3):
        for dx in range(3):
            w5[dy * 3 + dx] = W5e[:, :, dy, dx].T
    sh["w5"] = np.ascontiguousarray(w5.transpose(1, 0, 2)).astype(bf)   # (64,9,32)
    # ihc2 dy-stacked (stride 2), x H_SCALE so hst = H_SCALE * h
    Wi2 = np.asarray(inputs["Wi2"], np.float32) * H_SCALE
    wi2a = np.zeros((3, 64, 64), np.float32)
    wi2b = np.zeros((3, 32, 64), np.float32)
    for dx in range(3):
        wi2a[dx, 0:32] = Wi2[:, :, 0, dx].T
        wi2a[dx, 32:64] = Wi2[:, :, 1, dx].T
        wi2b[dx] = Wi2[:, :, 2, dx].T
    sh["wi2a"] = np.ascontiguousarray(wi2a.transpose(1, 0, 2)).astype(bf)     # (64,3,64)
    sh["wi2ao"] = np.ascontiguousarray(
        wi2a[:, list(range(32, 64)) + list(range(32))].transpose(1, 0, 2)).astype(bf)
    wi2b2 = np.zeros((64, 3, 64), np.float32)
    wi2b2[32:64] = wi2b.transpose(1, 0, 2)
    wi2b2[0:32] = wi2b.transpose(1, 0, 2)
    sh["wi2b2"] = wi2b2.astype(bf)
    # in-hand FC pix-paired, fp8 x WIF_SCALE: (128,50,512)
    wif3 = np.asarray(inputs["Wif"], np.float32).reshape(64, 100, 512)
    wifp2 = np.concatenate([wif3[:, :50], wif3[:, 50:]], axis=0)  # (128,50,512)
    sh["wifp2"] = (wifp2 * WIF_SCALE).astype(e4m3)
    sh["wes2"] = (np.asarray(inputs["Wes2"], np.float32) / WES1_SCALE).astype(bf)   # (1024,512)
    # wdf: ihv rows compensate H_SCALE*WIF_SCALE
    wdf = np.asarray(inputs["Wdf"], np.float32).copy()
    wdf[512:1024] /= (H_SCALE * WIF_SCALE)
    sh["wdf"] = wdf.astype(bf)
    sh["ident"] = np.tile(np.eye(32, dtype=np.float32), (4, 1)).astype(bf).copy()  # (128,32)
    # wes1 [p][t][o] swizzle, fp8-e3m4 x WES1_SCALE
    wes1 = np.asarray(inputs["Wes1"], np.float32)          # (16384, 1024)
    sh["wes1"] = (np.ascontiguousarray(
        wes1.reshape(128, 128, 1024).transpose(1, 0, 2)).reshape(128, 128 * 1024)
        * WES1_SCALE).astype(e3m4)

    # biases / tail constants (f32)
    b1e = np.repeat(np.asarray(inputs["b1"], np.float32), 2)
    b2e = np.repeat(np.asarray(inputs["b2"], np.float32), 2)
    b3e = np.repeat(np.asarray(inputs["b3"], np.float32), 2)
    b4e = np.repeat(np.asarray(inputs["b4"], np.float32), 2)
    b5e = np.repeat(np.asarray(inputs["b5"], np.float32), 2)
    sh["bc1"] = np.concatenate([b1e, b1e]).reshape(128, 1).copy()
    sh["bc2"] = b2e.reshape(128, 1).copy()
    sh["bc3"] = b3e.reshape(128, 2, order="F").copy()  # [p, mt]
    sh["bc4"] = b4e.reshape(64, 1).copy()
    sh["b5rep"] = np.tile(b5e, (BC, 1)).copy()                       # (32,32)
    sh["bi1c"] = np.tile(np.asarray(inputs["bi1"], np.float32), 4).reshape(128, 1).copy()
    sh["bi2c"] = (np.tile(np.asarray(inputs["bi2"], np.float32), 2).reshape(128, 1)
                  * H_SCALE).copy()
    sh["bes1t"] = (np.asarray(inputs["bes1"], np.float32).reshape(8, 128).T
                   * WES1_SCALE).copy()                              # (128,8)
    sh["bes2t"] = np.asarray(inputs["bes2"], np.float32).reshape(4, 128).T.copy()   # (128,4)
    sh["bifrep"] = (np.tile(np.asarray(inputs["bif"], np.float32), (BC, 1))
                    * (H_SCALE * WIF_SCALE)).copy()                  # (32,512)
    sh["bdfrep"] = np.tile(np.asarray(inputs["bdf"], np.float32), (BC, 1)).copy()   # (32,528)
    kappa2 = np.asarray(inputs["kappa2"], np.float32)
    W2f = np.stack([np.roll(kappa2, g, axis=2) for g in range(2)], axis=1).reshape(4, 32)
    sh["w2rep"] = np.tile(W2f, (BC, 1, 1)).copy()                    # (32,4,32)
    sh["b2frep"] = np.tile(np.repeat(np.asarray(inputs["b2f"], np.float32), 2), (BC, 1)).copy()

    # per-core tensors
    obs2 = obs.reshape(B, 128, 128)  # [s][t][p] with k = t*128 + p
    img_flat = _pad_flat(patch[:, :1])
    ih_flat = _pad_flat(patch[:, 1:])

    def _im2col_quad(flat):
        # quad chunks: out[c][h][q][j][:] = im2col tap q of sample (4c + 2h + j)
        # (h = row-half {0,64}, j = col-group partner {0,64})
        nchunk = flat.shape[0] // 4
        out = np.empty((nchunk, 2, 9, 2, 42 * 42), np.float32)
        for c in range(nchunk):
            for h in range(2):
                for j in range(2):
                    s = 4 * c + 2 * h + j
                    for q in range(9):
                        off = (q // 3) * 42 + q % 3
                        out[c, h, q, j] = flat[s, off:off + 42 * 42]
        return out.astype(bf)

    per_core = []
    for c in range(NCORES):
        m = dict(sh)
        sl = slice(c * BC, (c + 1) * BC)
        m["obsT"] = np.ascontiguousarray(
            obs2[sl].transpose(2, 1, 0)).reshape(128, 128 * BC).astype(bf)
        m["imgc"] = _im2col_quad(img_flat[sl])
        m["ihc"] = _im2col_quad(ih_flat[sl])
        per_core.append(m)
    return per_core


# ---------------------------------------------------------------- bass build
def build(debug=(), reps=1, sim=False):
    nc = bacc.Bacc("TRN2", target_bir_lowering=False, debug=False, num_devices=NCORES)

    D = {}

    def din(name, shape, dt=bf16):
        D[name] = nc.dram_tensor(name, list(shape), dt, kind="ExternalInput")
        return D[name]

    obsT_d = din("obsT", (128, 128 * BC))
    wes1_d = din("wes1", (128, 128 * 1024), fp8e3)
    imgc_d = din("imgc", (BC // 4, 2, 9, 2, 42 * 42))
    ihc_d = din("ihc", (BC // 4, 2, 9, 2, 42 * 42))
    w1r_d = din("w1r", (128, 64))
    wi1r_d = din("wi1r", (128, 32))
    w2a_d = din("w2a", (128, 3, 128))
    w2ao_d = din("w2ao", (128, 3, 128))
    w2b2_d = din("w2b2", (128, 3, 128))
    w3_d = din("w3", (128, 18, 128))
    w4_d = din("w4", (128, 18, 64))
    w5_d = din("w5", (64, 9, 32))
    wi2a_d = din("wi2a", (64, 3, 64))
    wi2ao_d = din("wi2ao", (64, 3, 64))
    wi2b2_d = din("wi2b2", (64, 3, 64))
    wifp2_d = din("wifp2", (128, 50, 512), fp8e4)
    wes2_d = din("wes2", (1024, 512))
    wdf_d = din("wdf", (1024, 528))
    ident_d = din("ident", (128, 32))
    for nm, shp in [("bc1", (128, 1)), ("bc2", (128, 1)), ("bc3", (128, 2)),
                    ("bc4", (64, 1)), ("b5rep", (BC, 32)), ("bi1c", (128, 1)),
                    ("bi2c", (128, 1)), ("bes1t", (128, 8)), ("bes2t", (128, 4)),
                    ("bifrep", (BC, 512)), ("bdfrep", (BC, 528)),
                    ("w2rep", (BC, 4, 32)), ("b2frep", (BC, 4))]:
        din(nm, shp, f32)

    out_d = nc.dram_tensor("out", [BC, 4], f32, kind="ExternalOutput")

    dbg_handles = {}

    def dbg(name, shape, dt):
        dbg_handles[name] = nc.dram_tensor(name, list(shape), dt, kind="ExternalOutput")
        return dbg_handles[name]

    with tile.TileContext(nc) as tc:
        with tc.tile_pool(name="pw", bufs=1) as pw, \
             tc.tile_pool(name="pwif", bufs=2) as pwif, \
             tc.tile_pool(name="pes2", bufs=3) as pes2, \
             tc.tile_pool(name="psum", bufs=2, space="PSUM") as psp, \
             tc.tile_pool(name="pes1p", bufs=1, space="PSUM") as pes1p:

            _sc = [None]

            def mark(name):
                if _sc[0] is not None:
                    nc.leave_named_scope(_sc[0][0], _sc[0][1], False)
                    _sc[0] = None
                if name:
                    sid, _ = nc.enter_named_scope(name, False)
                    _sc[0] = (name, sid)

            # ---------- persistent weight tiles
            mark("wload")
            def ld(name, shape, src_ap, dt=bf16, pool=None):
                t = (pool or pw).tile(list(shape), dt, tag=name)
                nc.sync.dma_start(out=t[:], in_=src_ap)
                return t

            w1r_t = ld("w1r", (128, 64), w1r_d[:])
            wi1r_t = ld("wi1r", (128, 32), wi1r_d[:])
            bias_t = {}
            for nm, shp in [("bc1", (128, 1)), ("bi1c", (128, 1))]:
                bias_t[nm] = ld(nm, shp, D[nm][:], dt=f32)

            def load_deferred_weights():
                gw = {}
                gw["w3"] = ld("w3", (128, 18, 128), w3_d[:])
                gw["w4"] = ld("w4", (128, 18, 64), w4_d[:])
                gw["w5"] = ld("w5", (64, 9, 32), w5_d[:])
                gw["wi2a"] = ld("wi2a", (64, 3, 64), wi2a_d[:])
                gw["wi2ao"] = ld("wi2ao", (64, 3, 64), wi2ao_d[:])
                gw["wi2b2"] = ld("wi2b2", (64, 3, 64), wi2b2_d[:])
                gw["ident"] = ld("ident", (128, 32), ident_d[:])
                for nm, shp in [("bc3", (128, 2)),
                                ("bc4", (64, 1)), ("b5rep", (BC, 32)),
                                ("bi2c", (128, 1)), ("bes1t", (128, 8)), ("bes2t", (128, 4)),
                                ("bifrep", (BC, 512)), ("bdfrep", (BC, 528)),
                                ("w2rep", (BC, 4, 32)), ("b2frep", (BC, 4))]:
                    bias_t[nm] = ld(nm, shp, D[nm][:], dt=f32)
                return gw

            for rep in range(reps):
                # ---------- es1: own-samples full-K matmul (no collective).
                # obsT (bf16) stationary; wes1 chunks stream as fp8-e3m4 on the
                # SWDGE ring. kt%2 selects the PE column group {0,64}; group g
                # accumulates at psum partitions 64g..64g+32 of its own banks.
                obsT_t = pw.tile([128, 128, BC], bf16, tag="obsT")
                es1_acc = pes1p.tile([128, ES1_GROUPS, 2, 512], f32, tag="pes1")
                es1_w = {}
                KT_PER = 128 // ES1_CHUNKS  # kt per chunk

                def es1_dma(c):
                    wc = pes2.tile([128, KT_PER, 1024], fp8e3, tag="wes1c")
                    nc.gpsimd.dma_start(
                        out=wc[:],
                        in_=wes1_d[:, KT_PER * 1024 * c:KT_PER * 1024 * (c + 1)]
                        .rearrange("p (t o) -> p t o", o=1024))
                    es1_w[c] = wc

                def es1_mms(c):
                    wc = es1_w.pop(c)
                    for j in range(KT_PER):
                        kt = KT_PER * c + j
                        g = kt % ES1_GROUPS
                        tp = (0, 64 * g) if ES1_GROUPS > 1 else None
                        for nt in range(2):
                            nc.tensor.matmul(es1_acc[64 * g:64 * g + 32, g, nt, :],
                                             obsT_t[:, kt, :],
                                             wc[:, j, 512 * nt:512 * (nt + 1)],
                                             start=(kt < ES1_GROUPS),
                                             stop=(kt >= 128 - ES1_GROUPS),
                                             tile_position=tp)

                # Site-based scheduler: DMA trigger stays 2-3 chunks ahead of MM
                # consumption. Emission-order invariant: mms(c-2) precedes dma(c).
                es1_sched = {"dma": 0, "mm": 0}

                def es1_site(nmm=1):
                    for _ in range(max(nmm, 1)):
                        if nmm and es1_sched["mm"] < ES1_CHUNKS and es1_sched["mm"] + 2 <= es1_sched["dma"]:
                            es1_mms(es1_sched["mm"])
                            es1_sched["mm"] += 1
                        if es1_sched["dma"] < ES1_CHUNKS and es1_sched["dma"] < es1_sched["mm"] + 3:
                            es1_dma(es1_sched["dma"])
                            es1_sched["dma"] += 1

                def es1_drain():
                    while es1_sched["mm"] < ES1_CHUNKS:
                        while es1_sched["dma"] < min(ES1_CHUNKS, es1_sched["mm"] + 3):
                            es1_dma(es1_sched["dma"])
                            es1_sched["dma"] += 1
                        es1_mms(es1_sched["mm"])
                        es1_sched["mm"] += 1

                # in-hand FC weights (fp8): 4 chunks on the scalar HWDGE ring
                # (first two prefetch after c1 so startup DMA stays clear)
                wif_bufs = {}

                def wif_dma(q0, qn):
                    wifc = pwif.tile([128, 14, 512], fp8e4, tag="wifc")
                    nc.scalar.dma_start(out=wifc[:, 0:qn, :], in_=wifp2_d[:, q0:q0 + qn, :])
                    wif_bufs[q0] = wifc

                # ================= conv stage pools ============================
                with tc.tile_pool(name="pconv", bufs=1) as pc:
                    xihE = pc.tile([64, 16, 22, 22], bf16, tag="xihE")
                    xihO = pc.tile([64, 16, 22, 22], bf16, tag="xihO")
                    x1pE = pc.tile([128, 16, 22, 22], bf16, tag="x1pE")
                    x1pO = pc.tile([128, 16, 22, 22], bf16, tag="x1pO")
                    hst = pc.tile([128, 16, 10, 10], bf16, tag="hst")
                    h_lin2 = pc.tile([128, BC, 50], bf16, tag="h_lin2")
                    x2 = pc.tile([128, BC, 10, 10], bf16, tag="x2")
                    x3 = pc.tile([128, 2, BC, 8, 8], bf16, tag="x3")
                    x4 = pc.tile([64, BC, 3, 3], bf16, tag="x4")

                    # border zeroing (interiors written by conv evictions)
                    for t_, p0, p1_ in ((xihE, 0, 32), (xihO, 32, 64), (x1pE, 0, 64), (x1pO, 64, 128)):
                        nc.vector.memset(t_[p0:p1_, :, 0:1, :], 0.0)
                        nc.vector.memset(t_[p0:p1_, :, 21:22, :], 0.0)
                        nc.vector.memset(t_[p0:p1_, :, :, 0:1], 0.0)
                        nc.vector.memset(t_[p0:p1_, :, :, 21:22], 0.0)

                    def quad_dma(dram, t9, c):
                        # load both row-halves of quad chunk c
                        for h in range(2):
                            nc.sync.dma_start(
                                out=t9[64 * h:64 * h + 9, :, :].rearrange("p j f -> p (j f)"),
                                in_=dram[c, h].rearrange("q j f -> q (j f)"))

                    with tc.tile_pool(name="pim", bufs=2) as pim, \
                         tc.tile_pool(name="pq", bufs=1, space="PSUM") as pq:

                        # ---------- ihc1: im2col, stride 2, quad row+col tiling
                        mark("ihc1")
                        for chunk in range(8):
                            t9 = pim.tile([128, 2, 42 * 42], bf16, tag="t9")
                            quad_dma(ihc_d, t9, chunk)
                            if chunk == 1:
                                # kick off the (large) obs stream once the first
                                # conv chunk is in flight
                                nc.gpsimd.dma_start(
                                    out=obsT_t[:],
                                    in_=obsT_d[:].rearrange("p (t s) -> p t s", s=BC))
                            pp = pq.tile([64, 2, 512], f32, tag="q")
                            for h in range(2):
                                for j in range(2):
                                    nc.tensor.matmul(
                                        pp[32 * j:32 * (j + 1), h, 0:400]
                                        .rearrange("p (y x) -> p y x", x=20),
                                        wi1r_t[64 * h:64 * h + 9, :],
                                        t9[64 * h:64 * h + 9, j, :]
                                        .rearrange("p (y x) -> p y x", x=42)[:, 0:40:2, 0:40:2],
                                        start=True, stop=True, tile_position=(64 * h, 32 * j))
                            m0 = 2 * chunk
                            nc.scalar.activation(
                                xihE[0:32, m0:m0 + 2, 1:21, 1:21],
                                pp[0:32, :, 0:400].rearrange("p h (y x) -> p h y x", x=20),
                                AF.Relu, bias=bias_t["bi1c"][0:32, 0:1])
                            nc.vector.tensor_scalar(
                                xihO[32:64, m0:m0 + 2, 1:21, 1:21],
                                pp[32:64, :, 0:400].rearrange("p h (y x) -> p h y x", x=20),
                                bias_t["bi1c"][32:64, 0:1], 0.0, ALU.add, ALU.max)
                            # per-chunk dy-stack shift copies (overlap the loop)
                            nc.scalar.dma_start(out=xihE[32:64, m0:m0 + 2, 0:21, :],
                                                in_=xihE[0:32, m0:m0 + 2, 1:22, :])
                            nc.scalar.dma_start(out=xihO[0:32, m0:m0 + 2, 0:21, :],
                                                in_=xihO[32:64, m0:m0 + 2, 1:22, :])
                            es1_site(nmm=0)

                        # conv2 weights load during the front window so c2 can
                        # interleave into the c1 chunk loop
                        w2a_t = ld("w2a", (128, 3, 128), w2a_d[:])
                        w2ao_t = ld("w2ao", (128, 3, 128), w2ao_d[:])
                        w2b2_t = ld("w2b2", (128, 3, 128), w2b2_d[:])
                        bias_t["bc2"] = ld("bc2", (128, 1), D["bc2"][:], dt=f32)

                        def c2_pair(pair):
                            m = pair
                            ppE = psp.tile([128, 20, 20], f32, tag="mm")
                            ppO = psp.tile([128, 20, 20], f32, tag="mm")
                            for dx in range(3):
                                nc.tensor.matmul(ppE[:], w2a_t[:, dx, :], x1pE[:, m, 0:20, dx:dx + 20],
                                                 start=(dx == 0), stop=False)
                            for dx in range(3):
                                nc.tensor.matmul(ppO[:], w2ao_t[:, dx, :], x1pO[:, m, 0:20, dx:dx + 20],
                                                 start=(dx == 0), stop=False)
                            for dx in range(3):
                                nc.tensor.matmul(ppE[:], w2b2_t[64:128, dx, :],
                                                 x1pE[64:128, m, 1:21, dx:dx + 20],
                                                 start=False, stop=(dx == 2))
                                nc.tensor.matmul(ppO[:], w2b2_t[0:64, dx, :],
                                                 x1pO[0:64, m, 1:21, dx:dx + 20],
                                                 start=False, stop=(dx == 2))
                            for s, pp in ((2 * pair, ppE), (2 * pair + 1, ppO)):
                                t2 = pc.tile([128, 20, 20], bf16, tag="c2e")
                                nc.scalar.activation(t2[:], pp[:], AF.Relu, bias=bias_t["bc2"][:, 0:1])
                                h2 = pc.tile([128, 20, 10], bf16, tag="c2h")
                                nc.vector.tensor_tensor(h2[:], t2[:, :, 0:20:2], t2[:, :, 1:20:2], ALU.max)
                                nc.vector.tensor_tensor(x2[:, s, :, :], h2[:, 0:20:2, :], h2[:, 1:20:2, :], ALU.max)
                            es1_site()

                        # ---------- c1: im2col, stride 1, quad tiling, fused pool
                        mark("c1")
                        for chunk in range(8):
                            t9 = pim.tile([128, 2, 42 * 42], bf16, tag="t9")
                            quad_dma(imgc_d, t9, chunk)
                            m0 = 2 * chunk
                            for piece in range(4):  # 10 conv rows -> 5 pooled rows each
                                r0c = piece * 10
                                pp = pq.tile([128, 2, 512], f32, tag="q")
                                for h in range(2):
                                    for j in range(2):
                                        nc.tensor.matmul(
                                            pp[64 * j:64 * (j + 1), h, 0:400]
                                            .rearrange("p (y x) -> p y x", x=40),
                                            w1r_t[64 * h:64 * h + 9, :],
                                            t9[64 * h:64 * h + 9, j, :]
                                            .rearrange("p (y x) -> p y x", x=42)[:, r0c:r0c + 10, 0:40],
                                            start=True, stop=True, tile_position=(64 * h, 64 * j))
                                ppv = pp[:, :, 0:400].rearrange("p h (y x) -> p h y x", x=40)
                                tt = pim.tile([128, 2, 10, 40], bf16, tag="c1e")
                                nc.scalar.activation(tt[:], ppv, AF.Relu, bias=bias_t["bc1"][:, 0:1])
                                hh = pim.tile([128, 2, 10, 20], bf16, tag="c1h")
                                nc.vector.tensor_tensor(hh[:], tt[:, :, :, 0:40:2], tt[:, :, :, 1:40:2], ALU.max)
                                r0 = 1 + 5 * piece
                                nc.vector.tensor_tensor(
                                    x1pE[0:64, m0:m0 + 2, r0:r0 + 5, 1:21],
                                    hh[0:64, :, 0:10:2, :], hh[0:64, :, 1:10:2, :], ALU.max)
                                nc.vector.tensor_tensor(
                                    x1pO[64:128, m0:m0 + 2, r0:r0 + 5, 1:21],
                                    hh[64:128, :, 0:10:2, :], hh[64:128, :, 1:10:2, :], ALU.max)
                                es1_site(nmm=(1 if piece % 2 == 1 else 0))
                            # per-chunk dy-stack shift copies (overlap the loop)
                            nc.scalar.dma_start(out=x1pE[64:128, m0:m0 + 2, 0:21, :],
                                                in_=x1pE[0:64, m0:m0 + 2, 1:22, :])
                            nc.scalar.dma_start(out=x1pO[0:64, m0:m0 + 2, 0:21, :],
                                                in_=x1pO[64:128, m0:m0 + 2, 1:22, :])
                    wif_dma(0, 14)
                    wif_dma(14, 12)
                    gw = load_deferred_weights()
                    w3_t, w4_t, w5_t = gw["w3"], gw["w4"], gw["w5"]
                    wi2a_t, wi2ao_t, wi2b2_t = gw["wi2a"], gw["wi2ao"], gw["wi2b2"]
                    ident_t = gw["ident"]

                    # -- es tail stages: the serial fold chain (DVE copies ->
                    # partition-shift DMA -> add) interleaves into ihc2; the PE
                    # stages (transposes + es2) run right after, before ihv.
                    catT = pw.tile([128, 8, BC], bf16, tag="catT")
                    esb = pw.tile([BC, 1024], bf16, tag="esb")
                    esT = pw.tile([128, 8, BC], bf16, tag="esT")
                    wdf_box = {}

                    def estail_stage(i):
                        if i == 0:
                            es1_drain()
                            wes2_t = pes2.tile([128, 8, 512], bf16, tag="wes1c", name="wes2t")
                            nc.sync.dma_start(out=wes2_t[:],
                                              in_=wes2_d[:].rearrange("(t p) o -> p t o", p=128))
                            wdf_t = pes2.tile([128, 8, 528], bf16, tag="wes1c", name="wdft")
                            nc.sync.dma_start(out=wdf_t[:],
                                              in_=wdf_d[:].rearrange("(t p) o -> p t o", p=128))
                            wdf_box["wes2"] = wes2_t
                            wdf_box["wdf"] = wdf_t
                        elif i == 1:
                            es1hi = pw.tile([96, 1024], bf16, tag="es1hi")
                            nc.vector.tensor_copy(es1hi[64:96, :], es1_acc[64:96, 1, :, :]
                                                  .rearrange("p a b -> p (a b)"))
                            esblo = pw.tile([BC, 2, 1024], bf16, tag="esblo")
                            nc.vector.tensor_copy(esblo[:, 0, :], es1_acc[0:32, 0, :, :]
                                                  .rearrange("p a b -> p (a b)"))
                            nc.scalar.dma_start(out=esblo[:, 1, :], in_=es1hi[64:96, :])
                            wdf_box["esblo"] = esblo
                        elif i == 2:
                            esblo = wdf_box["esblo"]
                            nc.vector.tensor_tensor(esb[:], esblo[:, 0, :], esblo[:, 1, :], ALU.add)
                        elif i in (3, 4):
                            for t in range(4 * (i - 3), 4 * (i - 2)):
                                pt = pssm.tile([128, BC], bf16, tag="sm")
                                nc.tensor.transpose(pt[:], esb[:, 128 * t:128 * (t + 1)],
                                                    ident_t[0:32, :])
                                nc.vector.tensor_scalar(esT[:, t, :], pt[:],
                                                        bias_t["bes1t"][:, t:t + 1],
                                                        0.0, ALU.add, ALU.max)
                        elif i in (5, 6):
                            for mt in range(2 * (i - 5), 2 * (i - 4)):
                                pp = pssm.tile([128, BC], f32, tag="sm")
                                for t in range(8):
                                    nc.tensor.matmul(pp[:],
                                                     wdf_box["wes2"][:, t, mt * 128:(mt + 1) * 128],
                                                     esT[:, t, :], start=(t == 0), stop=(t == 7))
                                nc.vector.tensor_scalar(catT[:, mt, :], pp[:],
                                                        bias_t["bes2t"][:, mt:mt + 1],
                                                        0.0, ALU.add, ALU.max)

                    # ---------- ihc2: dy-stacked stride-2 conv, E/O interleaved,
                    # two pairs per matmul (N=200)
                    mark("ihc2")
                    for g in range(8):
                        p0 = 2 * g
                        pp = psp.tile([128, 2, 10, 10], f32, tag="mm")
                        for dx in range(3):
                            nc.tensor.matmul(pp[0:64, :, :, :], wi2a_t[:, dx, :],
                                             xihE[0:64, p0:p0 + 2, 0:20:2, dx:dx + 20:2],
                                             start=(dx == 0), stop=False, tile_position=(0, 0))
                            nc.tensor.matmul(pp[64:128, :, :, :], wi2ao_t[:, dx, :],
                                             xihO[0:64, p0:p0 + 2, 0:20:2, dx:dx + 20:2],
                                             start=(dx == 0), stop=False, tile_position=(0, 64))
                        for dx in range(3):
                            nc.tensor.matmul(pp[0:64, :, :, :], wi2b2_t[32:64, dx, :],
                                             xihE[32:64, p0:p0 + 2, 1:21:2, dx:dx + 20:2],
                                             start=False, stop=(dx == 2), tile_position=(32, 0))
                            nc.tensor.matmul(pp[64:128, :, :, :], wi2b2_t[0:32, dx, :],
                                             xihO[0:32, p0:p0 + 2, 1:21:2, dx:dx + 20:2],
                                             start=False, stop=(dx == 2), tile_position=(0, 64))
                        nc.scalar.activation(hst[:, p0:p0 + 2, :, :], pp[:], AF.Relu,
                                             bias=bias_t["bi2c"][:, 0:1])
                        if g in (2, 5):
                            es1_site()
                    # h_lin2[(pixgroup, ch), s, q] (bf16, holds H_SCALE * h)
                    nc.sync.dma_start(out=h_lin2[0:64, 0:32:2, :],
                                      in_=hst[0:64, :, 0:5, :].rearrange("p k a b -> p k (a b)"))
                    nc.sync.dma_start(out=h_lin2[0:64, 1:32:2, :],
                                      in_=hst[64:128, :, 0:5, :].rearrange("p k a b -> p k (a b)"))
                    nc.sync.dma_start(out=h_lin2[64:128, 0:32:2, :],
                                      in_=hst[0:64, :, 5:10, :].rearrange("p k a b -> p k (a b)"))
                    nc.sync.dma_start(out=h_lin2[64:128, 1:32:2, :],
                                      in_=hst[64:128, :, 5:10, :].rearrange("p k a b -> p k (a b)"))
                    # fp8 pixel-major copy for the DoubleRow ihv matmuls
                    hq = pc.tile([128, 50, BC], fp8e4, tag="hq")
                    nc.vector.tensor_copy(hq[:], h_lin2[:].rearrange("p s q -> p q s"))
                    if "dbg_hlin" in debug:
                        nc.sync.dma_start(out=dbg("dbg_hlin", (128, BC, 50), bf16)[:], in_=h_lin2[:])

                    with tc.tile_pool(name="pssm", bufs=2, space="PSUM") as pssm:
                        # ---------- c2: dy-stacked 3x3 conv + pool, with the es
                        # tail stages interleaved (their serial fold chain hides
                        # behind c2's matmuls)
                        mark("c2")
                        for pair in range(16):
                            c2_pair(pair)
                            if pair >= 2 and pair % 2 == 0 and pair // 2 - 1 < 7:
                                estail_stage(pair // 2 - 1)
                        if "dbg_x2" in debug:
                            nc.sync.dma_start(out=dbg("dbg_x2", (128, BC, 10, 10), bf16)[:], in_=x2[:])
                        if "dbg_esT" in debug:
                            nc.sync.dma_start(out=dbg("dbg_esT", (128, 8, BC), bf16)[:], in_=esT[:])

                        # ---------- ihv: 25 fp8 DoubleRow matmuls (K=256 each)
                        mark("ihv")
                        p_ihv = pssm.tile([BC, 512], f32, tag="sm")
                        for q0, qn in [(0, 14), (14, 12), (26, 12), (38, 12)]:
                            if q0 not in wif_bufs:
                                wif_dma(q0, qn)
                            wifc = wif_bufs.pop(q0)
                            for k in range(qn // 2):
                                qg = q0 + 2 * k
                                nc.tensor.matmul(p_ihv[:], hq[:, qg:qg + 2, :],
                                                 wifc[:, 2 * k:2 * k + 2, :],
                                                 start=(qg == 0), stop=(qg == 48),
                                                 perf_mode=DR)
                        ihv_f = pc.tile([BC, 512], f32, tag="ihv_f")
                        nc.vector.tensor_tensor(ihv_f[:], p_ihv[:], bias_t["bifrep"][:], ALU.add)
                        ihvb = pc.tile([BC, 512], bf16, tag="ihvb")
                        nc.vector.tensor_scalar(ihvb[:], ihv_f[:], 0.0, None, ALU.max)
                        if "dbg_ihv" in debug:
                            nc.sync.dma_start(out=dbg("dbg_ihv", (BC, 512), bf16)[:], in_=ihvb[:])

                        for k in range(4):
                            pt = pssm.tile([128, BC], bf16, tag="sm")
                            nc.tensor.transpose(pt[:], ihvb[:, 128 * k:128 * (k + 1)], ident_t[0:32, :])
                            nc.vector.tensor_copy(catT[:, 4 + k, :], pt[:])
                        if "dbg_catT" in debug:
                            nc.sync.dma_start(out=dbg("dbg_catT", (128, 8, BC), bf16)[:], in_=catT[:])

                        # df emitted mid-c3 (after catT's ihv slots settle)
                        def df_stage():
                            pdf1 = pssm.tile([BC, 512], f32, tag="sm")
                            pdf2 = pssm.tile([BC, 16], f32, tag="sm")
                            for t in range(8):
                                nc.tensor.matmul(pdf1[:], catT[:, t, :], wdf_box["wdf"][:, t, 0:512],
                                                 start=(t == 0), stop=(t == 7))
                            for t in range(8):
                                nc.tensor.matmul(pdf2[:], catT[:, t, :], wdf_box["wdf"][:, t, 512:528],
                                                 start=(t == 0), stop=(t == 7))
                            wb_sb = pc.tile([BC, 528], f32, tag="wb_sb")
                            nc.vector.tensor_tensor(wb_sb[:, 0:512], pdf1[:],
                                                    bias_t["bdfrep"][:, 0:512], ALU.add)
                            nc.vector.tensor_tensor(wb_sb[:, 512:528], pdf2[:],
                                                    bias_t["bdfrep"][:, 512:528], ALU.add)
                            if "dbg_wb" in debug:
                                nc.sync.dma_start(out=dbg("dbg_wb", (BC, 528), f32)[:], in_=wb_sb[:])
                            return wb_sb

                        # ---------- c3
                        mark("c3")
                        wb_sb = None
                        for mt in range(2):
                            for sg in range(4):
                                pp = psp.tile([128, 8, 8, 8], f32, tag="mm")
                                first = True
                                for dy in range(3):
                                    for dx in range(3):
                                        nc.tensor.matmul(pp[:], w3_t[:, mt * 9 + dy * 3 + dx, :],
                                                         x2[:, sg * 8:(sg + 1) * 8, dy:dy + 8, dx:dx + 8],
                                                         start=first, stop=(dy == 2 and dx == 2))
                                        first = False
                                nc.scalar.activation(x3[:, mt, sg * 8:(sg + 1) * 8, :, :], pp[:],
                                                     AF.Relu, bias=bias_t["bc3"][:, mt:mt + 1])
                                if mt == 0 and sg == 2:
                                    wb_sb = df_stage()
                        if "dbg_x3" in debug:
                            nc.sync.dma_start(out=dbg("dbg_x3", (128, 2, BC, 8, 8), bf16)[:], in_=x3[:])

                        # ---------- c4 + pool
                        mark("c4")
                        for sg in range(4):
                            pp = psp.tile([64, 8, 6, 6], f32, tag="mm")
                            first = True
                            for kt in range(2):
                                for dy in range(3):
                                    for dx in range(3):
                                        nc.tensor.matmul(pp[:], w4_t[:, kt * 9 + dy * 3 + dx, :],
                                                         x3[:, kt, sg * 8:(sg + 1) * 8, dy:dy + 6, dx:dx + 6],
                                                         start=first, stop=(kt == 1 and dy == 2 and dx == 2))
                                        first = False
                            t4 = pc.tile([64, 8, 6, 6], bf16, tag="c4e")
                            nc.scalar.activation(t4[:], pp[:], AF.Relu, bias=bias_t["bc4"][:, 0:1])
                            h4 = pc.tile([64, 8, 6, 3], bf16, tag="c4h")
                            nc.vector.tensor_tensor(h4[:], t4[:, :, :, 0:6:2], t4[:, :, :, 1:6:2], ALU.max)
                            nc.vector.tensor_tensor(x4[:, sg * 8:(sg + 1) * 8, :, :],
                                                    h4[:, :, 0:6:2, :], h4[:, :, 1:6:2, :], ALU.max)

                        # ---------- c5 (batch-major out: samples on partitions)
                        mark("c5")
                        pp5 = pssm.tile([BC, 32], f32, tag="sm")
                        for q in range(9):
                            dy, dx = divmod(q, 3)
                            nc.tensor.matmul(pp5[:], x4[:, :, dy, dx], w5_t[:, q, :],
                                             start=(q == 0), stop=(q == 8))
                        xs_t = pc.tile([BC, 16, 2], f32, tag="xs")
                        xs_p = pc.tile([BC, 16, 2], f32, tag="xs_p")
                        nc.vector.tensor_tensor(xs_p[:], pp5[:].rearrange("p (a b) -> p a b", b=2),
                                                bias_t["b5rep"][:].rearrange("p (a b) -> p a b", b=2), ALU.add)
                        nc.vector.tensor_scalar(xs_t[:], xs_p[:], 0.0, None, ALU.max)
                        xg1 = pc.tile([BC, 16, 2], f32, tag="xg1")
                        nc.vector.tensor_copy(xg1[:], xs_t[:, :, ::-1])
                        if "dbg_xf" in debug:
                            nc.sync.dma_start(out=dbg("dbg_xf", (BC, 16, 2), f32)[:], in_=xs_t[:])

                        # ---------- dynamic 1x1 group conv tail (all DVE)
                        mark("tail")
                        wbv = wb_sb[:, 0:512].rearrange("p (o j) -> p o j", j=32)
                        tmp0 = pc.tile([BC, 16, 32], f32, tag="tmp0")
                        tmp1 = pc.tile([BC, 16, 32], f32, tag="tmp1")
                        xb0 = xs_t[:].rearrange("p a b -> p (a b)").unsqueeze(1).broadcast_to((BC, 16, 32))
                        xb1 = xg1[:].rearrange("p a b -> p (a b)").unsqueeze(1).broadcast_to((BC, 16, 32))
                        nc.vector.tensor_mul(tmp0[:], wbv, xb0)
                        nc.vector.tensor_mul(tmp1[:], wbv, xb1)
                        featr = pc.tile([BC, 16, 2], f32, tag="featr")
                        f0 = pc.tile([BC, 16], f32, tag="f0")
                        f1 = pc.tile([BC, 16], f32, tag="f1")
                        nc.vector.tensor_reduce(f0[:], tmp0[:], mybir.AxisListType.X, ALU.add)
                        nc.vector.tensor_reduce(f1[:], tmp1[:], mybir.AxisListType.X, ALU.add)
                        nc.vector.tensor_tensor(featr[:, :, 0], f0[:], wb_sb[:, 512:528], ALU.add)
                        nc.vector.tensor_tensor(featr[:, :, 1], f1[:], wb_sb[:, 512:528], ALU.add)
                        nc.vector.tensor_scalar(featr[:], featr[:], 0.0, None, ALU.max)
                        fb_ = featr[:].rearrange("p a b -> p (a b)").unsqueeze(1).broadcast_to((BC, 4, 32))
                        tmp2 = pc.tile([BC, 4, 32], f32, tag="tmp2")
                        nc.vector.tensor_mul(tmp2[:], bias_t["w2rep"][:], fb_)
                        o4_t = pc.tile([BC, 4], f32, tag="o4")
                        nc.vector.tensor_reduce(o4_t[:], tmp2[:], mybir.AxisListType.X, ALU.add)
                        outsb = pc.tile([BC, 4], f32, tag="outsb")
                        nc.vector.tensor_tensor(outsb[:], o4_t[:], bias_t["b2frep"][:], ALU.add)
                        nc.sync.dma_start(out=out_d[:], in_=outsb[:])
                        mark(None)

    nc.compile()
    return nc, dbg_handles


# ---------------------------------------------------------------- run
_CACHE = {}


def _get_module(debug=(), reps=1, sim=False):
    key = (tuple(sorted(debug)), reps, sim)
    if key not in _CACHE:
        _CACHE[key] = build(debug, reps, sim)
    return _CACHE[key]


def run(inputs, debug=()):
    nc, dbg_handles = _get_module(debug)
    in_maps = host_prep(inputs)
    res = run_bass_kernel_spmd(nc, in_maps, list(range(NCORES)))
    return res


def kernel(**inputs):
    res = run(inputs)
    out = np.concatenate([np.asarray(res.results[c]["out"], np.float32) for c in range(NCORES)], axis=0)
    return out.reshape(256, 2, 2)


# ---------------------------------------------------------------- timing
def make_runner(nc, in_maps):
    """Builds a reusable jitted executor for `nc` (mirrors
    bass2jax.run_bass_via_pjrt's multi-core path) with device-resident inputs.
    Returns a zero-arg callable that executes once and blocks."""
    import jax
    import numpy as _np
    from jax.sharding import Mesh, PartitionSpec
    from jax.experimental.shard_map import shard_map
    from concourse import bass2jax as b2j

    b2j.install_neuronx_cc_hook()
    n_cores = len(in_maps)
    partition_name = nc.partition_id_tensor.name if nc.partition_id_tensor else None
    in_names, out_names, out_avals, zero_outs = [], [], [], []
    for alloc in nc.m.functions[0].allocations:
        if not isinstance(alloc, mybir.MemoryLocationSet):
            continue
        name = alloc.memorylocations[0].name
        if alloc.kind == "ExternalInput":
            if name != partition_name:
                in_names.append(name)
        elif alloc.kind == "ExternalOutput":
            out_names.append(name)
            shape = tuple(alloc.tensor_shape)
            dtype = mybir.dt.np(alloc.dtype)
            out_avals.append(jax.core.ShapedArray(shape, dtype))
            zero_outs.append(_np.zeros(shape, dtype))
    n_params = len(in_names)
    n_outs = len(out_avals)
    all_in_names = list(in_names) + out_names
    if partition_name is not None:
        all_in_names.append(partition_name)

    def _body(*args):
        operands = list(args)
        if partition_name is not None:
            operands.append(b2j.partition_id_tensor())
        outs = b2j._bass_exec_p.bind(
            *operands,
            out_avals=tuple(out_avals),
            in_names=tuple(all_in_names),
            out_names=tuple(out_names),
            lowering_input_output_aliases=(),
            sim_require_finite=True,
            sim_require_nnan=True,
            nc=nc,
        )
        return tuple(outs)

    devices = jax.devices()[:n_cores]
    mesh = Mesh(_np.asarray(devices), ("core",))
    in_specs = (PartitionSpec("core"),) * (n_params + n_outs)
    out_specs = (PartitionSpec("core"),) * len(out_names)
    donate = tuple(range(n_params, n_params + n_outs))
    sharded = jax.jit(
        shard_map(_body, mesh=mesh, in_specs=in_specs, out_specs=out_specs,
                  check_rep=False),
        donate_argnums=donate, keep_unused=True)
    concat_in = [
        _np.concatenate([_np.asarray(in_maps[c][nm]) for c in range(n_cores)], axis=0)
        for nm in in_names
    ]
    from jax.sharding import NamedSharding
    shard = NamedSharding(mesh, PartitionSpec("core"))
    in_dev = [jax.device_put(a, shard) for a in concat_in]
    zshapes = [((n_cores * z.shape[0],) + z.shape[1:], z.dtype) for z in zero_outs]

    def call():
        zs = [jax.device_put(_np.zeros(s, d), shard) for s, d in zshapes]
        outs = sharded(*in_dev, *zs)
        jax.block_until_ready(outs)
        return outs

    return call


def time_kernel_reps(inputs, iters=8, reps=4):
    """Differential in-program repetition timing with PAIRED alternation:
    builds reps=1 and reps=N modules, alternates r1/rN calls so slow drift in
    the shared box cancels, and reports the median paired difference /(N-1)."""
    import time
    in_maps = host_prep(inputs)
    nc1, _ = _get_module((), 1)
    call1 = make_runner(nc1, in_maps)
    ncN, _ = _get_module((), reps)
    callN = make_runner(ncN, in_maps)
    call1(); callN(); call1(); callN()
    diffs, t1s, tNs = [], [], []
    for _ in range(iters):
        t0 = time.perf_counter()
        call1()
        t1 = time.perf_counter()
        callN()
        t2 = time.perf_counter()
        t1s.append(t1 - t0)
        tNs.append(t2 - t1)
        diffs.append((t2 - t1) - (t1 - t0))
    import numpy as _np
    med = _np.median(diffs)
    return med / (reps - 1) * 1e9, _np.median(t1s) * 1e9, _np.median(tNs) * 1e9


def time_kernel(inputs, iters=10):
    """Returns (best_ns, floor_ns): wall time of one kernel execution and of a
    null kernel through the same dispatch path."""
    import time
    nc, _ = _get_module(())
    in_maps = host_prep(inputs)
    call = make_runner(nc, in_maps)
    call()
    ts = []
    for _ in range(iters):
        t0 = time.perf_counter()
        call()
        ts.append(time.perf_counter() - t0)
    best = min(ts)

    # null kernel floor
    key = "_null"
    if key not in _CACHE:
        ncn = bacc.Bacc("TRN2", target_bir_lowering=False, debug=False, num_devices=NCORES)
        xi = ncn.dram_tensor("x", [128, 4], f32, kind="ExternalInput")
        yo = ncn.dram_tensor("y", [128, 4], f32, kind="ExternalOutput")
        with tile.TileContext(ncn) as tcn:
            with tcn.tile_pool(name="p", bufs=1) as pool:
                t = pool.tile([128, 4], f32)
                ncn.sync.dma_start(out=t[:], in_=xi[:])
                ncn.sync.dma_start(out=yo[:], in_=t[:])
        ncn.compile()
        _CACHE[key] = ncn
    ncn = _CACHE[key]
    calln = make_runner(ncn, [{"x": np.zeros((128, 4), np.float32)}] * NCORES)
    calln()
    tn = []
    for _ in range(iters):
        t0 = time.perf_counter()
        calln()
        tn.append(time.perf_counter() - t0)
    floor = min(tn)
    return best * 1e9, floor * 1e9


# revision 32
# speedup vs baseline: 1.1830x; 1.1830x over previous
# Trainium2 Bass kernel for nn_EquShiftQ2DF3P40 (group-equivariant CNN + dynamic filter).
#
# Sharding: batch 256 -> 32 samples/core on 8 cores, ALL weights replicated
# (collectives measured far slower than the redundant HBM traffic here).
#
# Optimizations over the original baseline (352us -> ~117us marginal):
# - Wes1 streams as fp8-e3m4 (x256 host scale, folded into bes1t/Wes2):
#   16.7MB instead of 33.5MB on the dominant weight stream. The es1 matmuls
#   are mixed-dtype: bf16 stationary obsT x fp8e3 moving weight chunks
#   (HW-validated exact; final rel err ~1.46e-2 vs the 2e-2 gate).
# - in-hand FC (6400->512) runs as 25 fp8-e4m3 DoubleRow matmuls (K=256
#   per MM); wifp2 streams fp8 (3.3MB). Scales (h x16 via wi2/bi2, wif x64)
#   fold into bifrep/wdf host-side - zero extra device ops.
# - conv1/ihc1 run 4 samples concurrently per piece via 2 row groups x
#   2 col groups (tile_position quads) - PE-array tiling overlaps the
#   K=9 matmuls on hardware (not visible in the cost-model sim).
# - c2's dy2 (K=64) matmuls of an even/odd sample pair are emitted
#   interleaved on disjoint row halves; ihc2 batches two sample-pairs per
#   matmul (N=200) with E/O col groups interleaved.
# - conv weights are pre-swizzled on host to partition-major so their loads
#   are contiguous descriptors; per-chunk dy-stack shift copies overlap the
#   conv loops on the scalar HWDGE ring.
# - the es tail is SPLIT INTO STAGES interleaved into the c2 loop (fold
#   chain: DVE evictions -> partition-shift DMA -> add; then transposes +
#   es2 matmuls), and the dynamic-filter matmuls run mid-c3, so the
#   post-conv critical path is just c4/c5 + the DVE tail.
# - NOTE (measured): merging c2 into the c1 chunk loop, or hoisting the
#   whole es tail as one block before c3, both REGRESSED (+24-40us) -
#   phase-level emission order is fragile because PE executes its queue
#   in order and serial non-PE chains stall it.
import numpy as np
import ml_dtypes

import concourse.bacc as bacc
import concourse.mybir as mybir
from concourse.bass_utils import run_bass_kernel_spmd
from concourse import tile
import bass_rust

f32 = mybir.dt.float32
bf16 = mybir.dt.bfloat16
fp8e3 = mybir.dt.float8e3
fp8e4 = mybir.dt.float8e4
AF = mybir.ActivationFunctionType
ALU = mybir.AluOpType
DR = mybir.MatmulPerfMode.DoubleRow
bf = ml_dtypes.bfloat16
e3m4 = ml_dtypes.float8_e3m4
e4m3 = ml_dtypes.float8_e4m3

NCORES = 8
BC = 32  # samples per core
ES1_GROUPS = 2  # PE column-tile groups for es1 (cols {0,64}, own psum banks)
WES1_SCALE = 256.0   # Wes1 fp8 scale; folded into bes1t (x) and Wes2 (/)
H_SCALE = 16.0       # in-hand conv2 output scale (wi2*, bi2c x16)
WIF_SCALE = 64.0     # Wif fp8 scale; H_SCALE*WIF_SCALE folds into bifrep/wdf
ES1_CHUNKS = 16      # 8 kt per chunk, 1MB fp8 each


# ---------------------------------------------------------------- host prep
def _rot(x, g):
    return np.rot90(x, k=g, axes=(-2, -1))


def _sym(k):
    return 0.5 * (k + _rot(k, 2))


def _expand_tq(kappa):
    kappa = _sym(kappa)
    Co, Ci, kh, kw = kappa.shape
    W = np.stack([_rot(kappa, g) for g in range(2)], axis=1)
    return W.reshape(Co * 2, Ci, kh, kw)


def _expand_qq(kappa):
    kappa = _sym(kappa)
    Co, Ci, F, kh, kw = kappa.shape
    W = np.stack([_rot(np.roll(kappa, g, axis=2), g) for g in range(F)], axis=1)
    return W.reshape(Co * F, Ci * F, kh, kw)


def _pad_flat(x):
    # (B,1,40,40) -> (B, 42*42 + 96) zero-padded flat images
    B = x.shape[0]
    p = np.zeros((B, 42, 42), np.float32)
    p[:, 1:41, 1:41] = x[:, 0]
    out = np.zeros((B, 42 * 42 + 96), np.float32)
    out[:, : 42 * 42] = p.reshape(B, -1)
    return out


def host_prep(inputs):
    """Returns per-core maps of numpy arrays keyed by dram names."""
    obs = np.asarray(inputs["obs_encoding"], np.float32)
    patch = np.asarray(inputs["patch"], np.float32)
    B = obs.shape[0]

    W1e = _expand_tq(np.asarray(inputs["k1"], np.float32))        # (64,1,3,3)
    W2e = _expand_qq(np.asarray(inputs["k2"], np.float32))        # (128,64,3,3)
    W3e = _expand_qq(np.asarray(inputs["k3"], np.float32))        # (256,128,3,3)
    W4e = _expand_qq(np.asarray(inputs["k4"], np.float32))        # (64,256,3,3)
    W5e = _expand_qq(np.asarray(inputs["k5"], np.float32))        # (32,64,3,3)

    sh = {}
    # conv1 / ihc1 im2col weights replicated at row-halves {0,64}
    w1r = np.zeros((128, 64), np.float32)
    wi1r = np.zeros((128, 32), np.float32)
    w19 = W1e[:, 0].reshape(64, 9).T  # (9, 64)
    wi19 = np.asarray(inputs["Wi1"], np.float32)[:, 0].reshape(32, 9).T
    for h in range(2):
        w1r[64 * h:64 * h + 9] = w19
        wi1r[64 * h:64 * h + 9] = wi19
    sh["w1r"] = w1r.astype(bf)
    sh["wi1r"] = wi1r.astype(bf)
    # conv2 dy-stacked: w2a (3,128,128) rows=(dy0 ci | dy1 ci); w2b (3,64,128) dy2
    w2a = np.zeros((3, 128, 128), np.float32)
    w2b = np.zeros((3, 64, 128), np.float32)
    for dx in range(3):
        w2a[dx, 0:64] = W2e[:, :, 0, dx].T
        w2a[dx, 64:128] = W2e[:, :, 1, dx].T
        w2b[dx] = W2e[:, :, 2, dx].T
    # partition-major swizzles for contiguous weight loads
    sh["w2a"] = np.ascontiguousarray(w2a.transpose(1, 0, 2)).astype(bf)       # (128,3,128)
    sh["w2ao"] = np.ascontiguousarray(
        w2a[:, list(range(64, 128)) + list(range(64))].transpose(1, 0, 2)).astype(bf)
    w2b2 = np.zeros((128, 3, 128), np.float32)   # both row halves hold dy2
    w2b2[64:128] = w2b.transpose(1, 0, 2)
    w2b2[0:64] = w2b.transpose(1, 0, 2)
    sh["w2b2"] = w2b2.astype(bf)
    # conv3: [p][(mt dy dx)][o]
    w3 = np.zeros((2, 3, 3, 128, 128), np.float32)
    for mt in range(2):
        for dy in range(3):
            for dx in range(3):
                w3[mt, dy, dx] = W3e[mt * 128:(mt + 1) * 128, :, dy, dx].T
    sh["w3"] = np.ascontiguousarray(w3.transpose(3, 0, 1, 2, 4)).reshape(128, 18, 128).astype(bf)
    # conv4: [p][(kt dy dx)][o]
    w4 = np.zeros((2, 3, 3, 128, 64), np.float32)
    for kt in range(2):
        for dy in range(3):
            for dx in range(3):
                w4[kt, dy, dx] = W4e[:, kt * 128:(kt + 1) * 128, dy, dx].T
    sh["w4"] = np.ascontiguousarray(w4.transpose(3, 0, 1, 2, 4)).reshape(128, 18, 64).astype(bf)
    # conv5: [p][q][o]
    w5 = np.zeros((9, 64, 32), np.float32)
    for dy in range(3):
        for dx in range(3):
            w5[dy * 3 + dx] = W5e[:, :, dy, dx].T
    sh["w5"] = np.ascontiguousarray(w5.transpose(1, 0, 2)).astype(bf)   # (64,9,32)
    # ihc2 dy-stacked (stride 2), x H_SCALE so hst = H_SCALE * h
    Wi2 = np.asarray(inputs["Wi2"], np.float32) * H_SCALE
    wi2a = np.zeros((3, 64, 64), np.float32)
    wi2b = np.zeros((3, 32, 64), np.float32)
    for dx in range(3):
        wi2a[dx, 0:32] = Wi2[:, :, 0, dx].T
        wi2a[dx, 32:64] = Wi2[:, :, 1, dx].T
        wi2b[dx] = Wi2[:, :, 2, dx].T
    sh["wi2a"] = np.ascontiguousarray(wi2a.transpose(1, 0, 2)).astype(bf)     # (64,3,64)
    sh["wi2ao"] = np.ascontiguousarray(
        wi2a[:, list(range(32, 64)) + list(range(32))].transpose(1, 0, 2)).astype(bf)
    wi2b2 = np.zeros((64, 3, 64), np.float32)
    wi2b2[32:64] = wi2b.transpose(1, 0, 2)
    wi2b2[0:32] = wi2b.transpose(1, 0, 2)
    sh["wi2b2"] = wi2b2.astype(bf)
    # in-hand FC pix-paired, fp8 x WIF_SCALE: (128,50,512)
    wif3 = np.asarray(inputs["Wif"], np.float32).reshape(64, 100, 512)
    wifp2 = np.concatenate([wif3[:, :50], wif3[:, 50:]], axis=0)  # (128,50,512)
    sh["wifp2"] = (wifp2 * WIF_SCALE).astype(e4m3)
    sh["wes2"] = (np.asarray(inputs["Wes2"], np.float32) / WES1_SCALE).astype(bf)   # (1024,512)
    # wdf: ihv rows compensate H_SCALE*WIF_SCALE
    wdf = np.asarray(inputs["Wdf"], np.float32).copy()
    wdf[512:1024] /= (H_SCALE * WIF_SCALE)
    sh["wdf"] = wdf.astype(bf)
    sh["ident"] = np.tile(np.eye(32, dtype=np.float32), (4, 1)).astype(bf).copy()  # (128,32)
    # wes1 [p][t][o] swizzle, fp8-e3m4 x WES1_SCALE
    wes1 = np.asarray(inputs["Wes1"], np.float32)          # (16384, 1024)
    sh["wes1"] = (np.ascontiguousarray(
        wes1.reshape(128, 128, 1024).transpose(1, 0, 2)).reshape(128, 128 * 1024)
        * WES1_SCALE).astype(e3m4)

    # biases / tail constants (f32)
    b1e = np.repeat(np.asarray(inputs["b1"], np.float32), 2)
    b2e = np.repeat(np.asarray(inputs["b2"], np.float32), 2)
    b3e = np.repeat(np.asarray(inputs["b3"], np.float32), 2)
    b4e = np.repeat(np.asarray(inputs["b4"], np.float32), 2)
    b5e = np.repeat(np.asarray(inputs["b5"], np.float32), 2)
    sh["bc1"] = np.concatenate([b1e, b1e]).reshape(128, 1).copy()
    sh["bc2"] = b2e.reshape(128, 1).copy()
    sh["bc3"] = b3e.reshape(128, 2, order="F").copy()  # [p, mt]
    sh["bc4"] = b4e.reshape(64, 1).copy()
    sh["b5rep"] = np.tile(b5e, (BC, 1)).copy()                       # (32,32)
    sh["bi1c"] = np.tile(np.asarray(inputs["bi1"], np.float32), 4).reshape(128, 1).copy()
    sh["bi2c"] = (np.tile(np.asarray(inputs["bi2"], np.float32), 2).reshape(128, 1)
                  * H_SCALE).copy()
    sh["bes1t"] = (np.asarray(inputs["bes1"], np.float32).reshape(8, 128).T
                   * WES1_SCALE).copy()                              # (128,8)
    sh["bes2t"] = np.asarray(inputs["bes2"], np.float32).reshape(4, 128).T.copy()   # (128,4)
    sh["bifrep"] = (np.tile(np.asarray(inputs["bif"], np.float32), (BC, 1))
                    * (H_SCALE * WIF_SCALE)).copy()                  # (32,512)
    sh["bdfrep"] = np.tile(np.asarray(inputs["bdf"], np.float32), (BC, 1)).copy()   # (32,528)
    kappa2 = np.asarray(inputs["kappa2"], np.float32)
    W2f = np.stack([np.roll(kappa2, g, axis=2) for g in range(2)], axis=1).reshape(4, 32)
    sh["w2rep"] = np.tile(W2f, (BC, 1, 1)).copy()                    # (32,4,32)
    sh["b2frep"] = np.tile(np.repeat(np.asarray(inputs["b2f"], np.float32), 2), (BC, 1)).copy()

    # per-core tensors
    obs2 = obs.reshape(B, 128, 128)  # [s][t][p] with k = t*128 + p
    img_flat = _pad_flat(patch[:, :1])
    ih_flat = _pad_flat(patch[:, 1:])

    def _im2col_quad(flat):
        # quad chunks: out[c][h][q][j][:] = im2col tap q of sample (4c + 2h + j)
        # (h = row-half {0,64}, j = col-group partner {0,64})
        nchunk = flat.shape[0] // 4
        out = np.empty((nchunk, 2, 9, 2, 42 * 42), np.float32)
        for c in range(nchunk):
            for h in range(2):
                for j in range(2):
                    s = 4 * c + 2 * h + j
                    for q in range(9):
                        off = (q // 3) * 42 + q % 3
                        out[c, h, q, j] = flat[s, off:off + 42 * 42]
        return out.astype(bf)

    per_core = []
    for c in range(NCORES):
        m = dict(sh)
        sl = slice(c * BC, (c + 1) * BC)
        m["obsT"] = np.ascontiguousarray(
            obs2[sl].transpose(2, 1, 0)).reshape(128, 128 * BC).astype(bf)
        m["imgc"] = _im2col_quad(img_flat[sl])
        m["ihc"] = _im2col_quad(ih_flat[sl])
        per_core.append(m)
    return per_core


# ---------------------------------------------------------------- bass build
def build(debug=(), reps=1, sim=False):
    nc = bacc.Bacc("TRN2", target_bir_lowering=False, debug=False, num_devices=NCORES)

    D = {}

    def din(name, shape, dt=bf16):
        D[name] = nc.dram_tensor(name, list(shape), dt, kind="ExternalInput")
        return D[name]

    obsT_d = din("obsT", (128, 128 * BC))
    wes1_d = din("wes1", (128, 128 * 1024), fp8e3)
    imgc_d = din("imgc", (BC // 4, 2, 9, 2, 42 * 42))
    ihc_d = din("ihc", (BC // 4, 2, 9, 2, 42 * 42))
    w1r_d = din("w1r", (128, 64))
    wi1r_d = din("wi1r", (128, 32))
    w2a_d = din("w2a", (128, 3, 128))
    w2ao_d = din("w2ao", (128, 3, 128))
    w2b2_d = din("w2b2", (128, 3, 128))
    w3_d = din("w3", (128, 18, 128))
    w4_d = din("w4", (128, 18, 64))
    w5_d = din("w5", (64, 9, 32))
    wi2a_d = din("wi2a", (64, 3, 64))
    wi2ao_d = din("wi2ao", (64, 3, 64))
    wi2b2_d = din("wi2b2", (64, 3, 64))
    wifp2_d = din("wifp2", (128, 50, 512), fp8e4)
    wes2_d = din("wes2", (1024, 512))
    wdf_d = din("wdf", (1024, 528))
    ident_d = din("ident", (128, 32))
    for nm, shp in [("bc1", (128, 1)), ("bc2", (128, 1)), ("bc3", (128, 2)),
                    ("bc4", (64, 1)), ("b5rep", (BC, 32)), ("bi1c", (128, 1)),
                    ("bi2c", (128, 1)), ("bes1t", (128, 8)), ("bes2t", (128, 4)),
                    ("bifrep", (BC, 512)), ("bdfrep", (BC, 528)),
                    ("w2rep", (BC, 4, 32)), ("b2frep", (BC, 4))]:
        din(nm, shp, f32)

    out_d = nc.dram_tensor("out", [BC, 4], f32, kind="ExternalOutput")

    dbg_handles = {}

    def dbg(name, shape, dt):
        dbg_handles[name] = nc.dram_tensor(name, list(shape), dt, kind="ExternalOutput")
        return dbg_handles[name]

    with tile.TileContext(nc) as tc:
        with tc.tile_pool(name="pw", bufs=1) as pw, \
             tc.tile_pool(name="pwif", bufs=3) as pwif, \
             tc.tile_pool(name="pes2", bufs=3) as pes2, \
             tc.tile_pool(name="psum", bufs=2, space="PSUM") as psp, \
             tc.tile_pool(name="pes1p", bufs=1, space="PSUM") as pes1p:

            _sc = [None]

            def mark(name):
                if _sc[0] is not None:
                    nc.leave_named_scope(_sc[0][0], _sc[0][1], False)
                    _sc[0] = None
                if name:
                    sid, _ = nc.enter_named_scope(name, False)
                    _sc[0] = (name, sid)

            # ---------- persistent weight tiles
            mark("wload")
            def ld(name, shape, src_ap, dt=bf16, pool=None):
                t = (pool or pw).tile(list(shape), dt, tag=name)
                nc.sync.dma_start(out=t[:], in_=src_ap)
                return t

            w1r_t = ld("w1r", (128, 64), w1r_d[:])
            wi1r_t = ld("wi1r", (128, 32), wi1r_d[:])
            bias_t = {}
            for nm, shp in [("bc1", (128, 1)), ("bi1c", (128, 1))]:
                bias_t[nm] = ld(nm, shp, D[nm][:], dt=f32)

            def load_deferred_weights():
                gw = {}
                gw["w3"] = ld("w3", (128, 18, 128), w3_d[:])
                gw["w4"] = ld("w4", (128, 18, 64), w4_d[:])
                gw["w5"] = ld("w5", (64, 9, 32), w5_d[:])
                for nm, shp in [("bc3", (128, 2)),
                                ("bc4", (64, 1)), ("b5rep", (BC, 32)),
                                ("bes1t", (128, 8)), ("bes2t", (128, 4)),
                                ("bifrep", (BC, 512)), ("bdfrep", (BC, 528)),
                                ("w2rep", (BC, 4, 32)), ("b2frep", (BC, 4))]:
                    bias_t[nm] = ld(nm, shp, D[nm][:], dt=f32)
                return gw

            for rep in range(reps):
                # ---------- es1: own-samples full-K matmul (no collective).
                # obsT (bf16) stationary; wes1 chunks stream as fp8-e3m4 on the
                # SWDGE ring. kt%2 selects the PE column group {0,64}; group g
                # accumulates at psum partitions 64g..64g+32 of its own banks.
                obsT_t = pw.tile([128, 128, BC], bf16, tag="obsT")
                es1_acc = pes1p.tile([128, ES1_GROUPS, 2, 512], f32, tag="pes1")
                es1_w = {}
                KT_PER = 128 // ES1_CHUNKS  # kt per chunk

                def es1_dma(c):
                    wc = pes2.tile([128, KT_PER, 1024], fp8e3, tag="wes1c")
                    nc.gpsimd.dma_start(
                        out=wc[:],
                        in_=wes1_d[:, KT_PER * 1024 * c:KT_PER * 1024 * (c + 1)]
                        .rearrange("p (t o) -> p t o", o=1024))
                    es1_w[c] = wc

                def es1_mms(c):
                    wc = es1_w.pop(c)
                    for j in range(KT_PER):
                        kt = KT_PER * c + j
                        g = kt % ES1_GROUPS
                        tp = (0, 64 * g) if ES1_GROUPS > 1 else None
                        for nt in range(2):
                            nc.tensor.matmul(es1_acc[64 * g:64 * g + 32, g, nt, :],
                                             obsT_t[:, kt, :],
                                             wc[:, j, 512 * nt:512 * (nt + 1)],
                                             start=(kt < ES1_GROUPS),
                                             stop=(kt >= 128 - ES1_GROUPS),
                                             tile_position=tp)

                # Site-based scheduler: DMA trigger stays 2-3 chunks ahead of MM
                # consumption. Emission-order invariant: mms(c-2) precedes dma(c).
                es1_sched = {"dma": 0, "mm": 0}

                def es1_site(nmm=1):
                    for _ in range(max(nmm, 1)):
                        if nmm and es1_sched["mm"] < ES1_CHUNKS and es1_sched["mm"] + 2 <= es1_sched["dma"]:
                            es1_mms(es1_sched["mm"])
                            es1_sched["mm"] += 1
                        if es1_sched["dma"] < ES1_CHUNKS and es1_sched["dma"] < es1_sched["mm"] + 3:
                            es1_dma(es1_sched["dma"])
                            es1_sched["dma"] += 1

                def es1_drain():
                    while es1_sched["mm"] < ES1_CHUNKS:
                        while es1_sched["dma"] < min(ES1_CHUNKS, es1_sched["mm"] + 3):
                            es1_dma(es1_sched["dma"])
                            es1_sched["dma"] += 1
                        es1_mms(es1_sched["mm"])
                        es1_sched["mm"] += 1

                # in-hand FC weights (fp8): 4 chunks on the scalar HWDGE ring
                # (first two prefetch after c1 so startup DMA stays clear)
                wif_bufs = {}

                def wif_dma(q0, qn):
                    wifc = pwif.tile([128, 14, 512], fp8e4, tag="wifc")
                    nc.scalar.dma_start(out=wifc[:, 0:qn, :], in_=wifp2_d[:, q0:q0 + qn, :])
                    wif_bufs[q0] = wifc

                # ================= conv stage pools ============================
                with tc.tile_pool(name="pconv", bufs=1) as pc:
                    xihE = pc.tile([64, 16, 22, 22], bf16, tag="xihE")
                    xihO = pc.tile([64, 16, 22, 22], bf16, tag="xihO")
                    x1pE = pc.tile([128, 16, 22, 22], bf16, tag="x1pE")
                    x1pO = pc.tile([128, 16, 22, 22], bf16, tag="x1pO")
                    hst = pc.tile([128, 16, 10, 10], bf16, tag="hst")
                    h_lin2 = pc.tile([128, BC, 50], bf16, tag="h_lin2")
                    x2 = pc.tile([128, BC, 10, 10], bf16, tag="x2")
                    x3 = pc.tile([128, 2, BC, 8, 8], bf16, tag="x3")
                    x4 = pc.tile([64, BC, 3, 3], bf16, tag="x4")

                    # border zeroing (interiors written by conv evictions)
                    for t_, p0, p1_ in ((xihE, 0, 32), (xihO, 32, 64), (x1pE, 0, 64), (x1pO, 64, 128)):
                        nc.vector.memset(t_[p0:p1_, :, 0:1, :], 0.0)
                        nc.vector.memset(t_[p0:p1_, :, 21:22, :], 0.0)
                        nc.vector.memset(t_[p0:p1_, :, :, 0:1], 0.0)
                        nc.vector.memset(t_[p0:p1_, :, :, 21:22], 0.0)

                    def quad_dma(dram, t9, c):
                        # load both row-halves of quad chunk c
                        for h in range(2):
                            nc.sync.dma_start(
                                out=t9[64 * h:64 * h + 9, :, :].rearrange("p j f -> p (j f)"),
                                in_=dram[c, h].rearrange("q j f -> q (j f)"))

                    with tc.tile_pool(name="pim", bufs=2) as pim, \
                         tc.tile_pool(name="pq", bufs=1, space="PSUM") as pq:

                        # ---------- ihc1: im2col, stride 2, quad row+col tiling
                        mark("ihc1")
                        for chunk in range(8):
                            t9 = pim.tile([128, 2, 42 * 42], bf16, tag="t9")
                            quad_dma(ihc_d, t9, chunk)
                            if chunk == 1:
                                # kick off the (large) obs stream once the first
                                # conv chunk is in flight
                                nc.gpsimd.dma_start(
                                    out=obsT_t[:],
                                    in_=obsT_d[:].rearrange("p (t s) -> p t s", s=BC))
                            pp = pq.tile([64, 2, 512], f32, tag="q")
                            for h in range(2):
                                for j in range(2):
                                    nc.tensor.matmul(
                                        pp[32 * j:32 * (j + 1), h, 0:400]
                                        .rearrange("p (y x) -> p y x", x=20),
                                        wi1r_t[64 * h:64 * h + 9, :],
                                        t9[64 * h:64 * h + 9, j, :]
                                        .rearrange("p (y x) -> p y x", x=42)[:, 0:40:2, 0:40:2],
                                        start=True, stop=True, tile_position=(64 * h, 32 * j))
                            m0 = 2 * chunk
                            nc.scalar.activation(
                                xihE[0:32, m0:m0 + 2, 1:21, 1:21],
                                pp[0:32, :, 0:400].rearrange("p h (y x) -> p h y x", x=20),
                                AF.Relu, bias=bias_t["bi1c"][0:32, 0:1])
                            nc.vector.tensor_scalar(
                                xihO[32:64, m0:m0 + 2, 1:21, 1:21],
                                pp[32:64, :, 0:400].rearrange("p h (y x) -> p h y x", x=20),
                                bias_t["bi1c"][32:64, 0:1], 0.0, ALU.add, ALU.max)
                            # per-chunk dy-stack shift copies (overlap the loop)
                            nc.scalar.dma_start(out=xihE[32:64, m0:m0 + 2, 0:21, :],
                                                in_=xihE[0:32, m0:m0 + 2, 1:22, :])
                            nc.scalar.dma_start(out=xihO[0:32, m0:m0 + 2, 0:21, :],
                                                in_=xihO[32:64, m0:m0 + 2, 1:22, :])
                            es1_site(nmm=0)

                        # conv2 weights load during the front window so c2 can
                        # interleave into the c1 chunk loop
                        w2a_t = ld("w2a", (128, 3, 128), w2a_d[:])
                        w2ao_t = ld("w2ao", (128, 3, 128), w2ao_d[:])
                        w2b2_t = ld("w2b2", (128, 3, 128), w2b2_d[:])
                        bias_t["bc2"] = ld("bc2", (128, 1), D["bc2"][:], dt=f32)
                        wi2a_t = ld("wi2a", (64, 3, 64), wi2a_d[:])
                        wi2ao_t = ld("wi2ao", (64, 3, 64), wi2ao_d[:])
                        wi2b2_t = ld("wi2b2", (64, 3, 64), wi2b2_d[:])
                        ident_t = ld("ident", (128, 32), ident_d[:])
                        bias_t["bi2c"] = ld("bi2c", (128, 1), D["bi2c"][:], dt=f32)

                        def c2_pair(pair):
                            m = pair
                            ppE = psp.tile([128, 20, 20], f32, tag="mm")
                            ppO = psp.tile([128, 20, 20], f32, tag="mm")
                            for dx in range(3):
                                nc.tensor.matmul(ppE[:], w2a_t[:, dx, :], x1pE[:, m, 0:20, dx:dx + 20],
                                                 start=(dx == 0), stop=False)
                            for dx in range(3):
                                nc.tensor.matmul(ppO[:], w2ao_t[:, dx, :], x1pO[:, m, 0:20, dx:dx + 20],
                                                 start=(dx == 0), stop=False)
                            for dx in range(3):
                                nc.tensor.matmul(ppE[:], w2b2_t[64:128, dx, :],
                                                 x1pE[64:128, m, 1:21, dx:dx + 20],
                                                 start=False, stop=(dx == 2))
                                nc.tensor.matmul(ppO[:], w2b2_t[0:64, dx, :],
                                                 x1pO[0:64, m, 1:21, dx:dx + 20],
                                                 start=False, stop=(dx == 2))
                            for s, pp in ((2 * pair, ppE), (2 * pair + 1, ppO)):
                                t2 = pc.tile([128, 20, 20], bf16, tag="c2e")
                                nc.scalar.activation(t2[:], pp[:], AF.Relu, bias=bias_t["bc2"][:, 0:1])
                                h2 = pc.tile([128, 20, 10], bf16, tag="c2h")
                                nc.vector.tensor_tensor(h2[:], t2[:, :, 0:20:2], t2[:, :, 1:20:2], ALU.max)
                                nc.vector.tensor_tensor(x2[:, s, :, :], h2[:, 0:20:2, :], h2[:, 1:20:2, :], ALU.max)
                            es1_site()

                        # ---------- c1: im2col, stride 1, quad tiling, fused pool
                        mark("c1")
                        for chunk in range(8):
                            t9 = pim.tile([128, 2, 42 * 42], bf16, tag="t9")
                            quad_dma(imgc_d, t9, chunk)
                            m0 = 2 * chunk
                            for piece in range(4):  # 10 conv rows -> 5 pooled rows each
                                r0c = piece * 10
                                pp = pq.tile([128, 2, 512], f32, tag="q")
                                for h in range(2):
                                    for j in range(2):
                                        nc.tensor.matmul(
                                            pp[64 * j:64 * (j + 1), h, 0:400]
                                            .rearrange("p (y x) -> p y x", x=40),
                                            w1r_t[64 * h:64 * h + 9, :],
                                            t9[64 * h:64 * h + 9, j, :]
                                            .rearrange("p (y x) -> p y x", x=42)[:, r0c:r0c + 10, 0:40],
                                            start=True, stop=True, tile_position=(64 * h, 64 * j))
                                ppv = pp[:, :, 0:400].rearrange("p h (y x) -> p h y x", x=40)
                                tt = pim.tile([128, 2, 10, 40], bf16, tag="c1e")
                                nc.scalar.activation(tt[:], ppv, AF.Relu, bias=bias_t["bc1"][:, 0:1])
                                hh = pim.tile([128, 2, 10, 20], bf16, tag="c1h")
                                nc.vector.tensor_tensor(hh[:], tt[:, :, :, 0:40:2], tt[:, :, :, 1:40:2], ALU.max)
                                r0 = 1 + 5 * piece
                                nc.vector.tensor_tensor(
                                    x1pE[0:64, m0:m0 + 2, r0:r0 + 5, 1:21],
                                    hh[0:64, :, 0:10:2, :], hh[0:64, :, 1:10:2, :], ALU.max)
                                nc.vector.tensor_tensor(
                                    x1pO[64:128, m0:m0 + 2, r0:r0 + 5, 1:21],
                                    hh[64:128, :, 0:10:2, :], hh[64:128, :, 1:10:2, :], ALU.max)
                                es1_site(nmm=(1 if piece % 2 == 1 else 0))
                            # per-chunk dy-stack shift copies (overlap the loop)
                            nc.scalar.dma_start(out=x1pE[64:128, m0:m0 + 2, 0:21, :],
                                                in_=x1pE[0:64, m0:m0 + 2, 1:22, :])
                            nc.scalar.dma_start(out=x1pO[0:64, m0:m0 + 2, 0:21, :],
                                                in_=x1pO[64:128, m0:m0 + 2, 1:22, :])
                    wif_dma(0, 14)
                    wif_dma(14, 12)
                    wif_dma(26, 12)
                    gw = load_deferred_weights()
                    w3_t, w4_t, w5_t = gw["w3"], gw["w4"], gw["w5"]

                    # -- es tail stages: the serial fold chain (DVE copies ->
                    # partition-shift DMA -> add) interleaves into ihc2; the PE
                    # stages (transposes + es2) run right after, before ihv.
                    catT = pw.tile([128, 8, BC], bf16, tag="catT")
                    esb = pw.tile([BC, 1024], bf16, tag="esb")
                    esT = pw.tile([128, 8, BC], bf16, tag="esT")
                    wdf_box = {}

                    def estail_stage(i):
                        if i == 0:
                            es1_drain()
                            wes2_t = pes2.tile([128, 8, 512], bf16, tag="wes1c", name="wes2t")
                            nc.sync.dma_start(out=wes2_t[:],
                                              in_=wes2_d[:].rearrange("(t p) o -> p t o", p=128))
                            wdf_t = pes2.tile([128, 8, 528], bf16, tag="wes1c", name="wdft")
                            nc.sync.dma_start(out=wdf_t[:],
                                              in_=wdf_d[:].rearrange("(t p) o -> p t o", p=128))
                            wdf_box["wes2"] = wes2_t
                            wdf_box["wdf"] = wdf_t
                        elif i == 1:
                            es1hi = pw.tile([96, 1024], bf16, tag="es1hi")
                            nc.vector.tensor_copy(es1hi[64:96, :], es1_acc[64:96, 1, :, :]
                                                  .rearrange("p a b -> p (a b)"))
                            esblo = pw.tile([BC, 2, 1024], bf16, tag="esblo")
                            nc.vector.tensor_copy(esblo[:, 0, :], es1_acc[0:32, 0, :, :]
                                                  .rearrange("p a b -> p (a b)"))
                            nc.scalar.dma_start(out=esblo[:, 1, :], in_=es1hi[64:96, :])
                            wdf_box["esblo"] = esblo
                        elif i == 2:
                            esblo = wdf_box["esblo"]
                            nc.vector.tensor_tensor(esb[:], esblo[:, 0, :], esblo[:, 1, :], ALU.add)
                        elif i in (3, 4):
                            for t in range(4 * (i - 3), 4 * (i - 2)):
                                pt = pssm.tile([128, BC], bf16, tag="sm")
                                nc.tensor.transpose(pt[:], esb[:, 128 * t:128 * (t + 1)],
                                                    ident_t[0:32, :])
                                nc.vector.tensor_scalar(esT[:, t, :], pt[:],
                                                        bias_t["bes1t"][:, t:t + 1],
                                                        0.0, ALU.add, ALU.max)
                        elif i in (5, 6):
                            for mt in range(2 * (i - 5), 2 * (i - 4)):
                                pp = pssm.tile([128, BC], f32, tag="sm")
                                for t in range(8):
                                    nc.tensor.matmul(pp[:],
                                                     wdf_box["wes2"][:, t, mt * 128:(mt + 1) * 128],
                                                     esT[:, t, :], start=(t == 0), stop=(t == 7))
                                nc.vector.tensor_scalar(catT[:, mt, :], pp[:],
                                                        bias_t["bes2t"][:, mt:mt + 1],
                                                        0.0, ALU.add, ALU.max)

                    # ---------- ihc2: dy-stacked stride-2 conv, E/O interleaved,
                    # four pairs per matmul (N=400)
                    mark("ihc2")
                    for g in range(4):
                        p0 = 4 * g
                        pp = psp.tile([128, 4, 10, 10], f32, tag="mm")
                        for dx in range(3):
                            nc.tensor.matmul(pp[0:64, :, :, :], wi2a_t[:, dx, :],
                                             xihE[0:64, p0:p0 + 4, 0:20:2, dx:dx + 20:2],
                                             start=(dx == 0), stop=False, tile_position=(0, 0))
                            nc.tensor.matmul(pp[64:128, :, :, :], wi2ao_t[:, dx, :],
                                             xihO[0:64, p0:p0 + 4, 0:20:2, dx:dx + 20:2],
                                             start=(dx == 0), stop=False, tile_position=(0, 64))
                        for dx in range(3):
                            nc.tensor.matmul(pp[0:64, :, :, :], wi2b2_t[32:64, dx, :],
                                             xihE[32:64, p0:p0 + 4, 1:21:2, dx:dx + 20:2],
                                             start=False, stop=(dx == 2), tile_position=(32, 0))
                            nc.tensor.matmul(pp[64:128, :, :, :], wi2b2_t[0:32, dx, :],
                                             xihO[0:32, p0:p0 + 4, 1:21:2, dx:dx + 20:2],
                                             start=False, stop=(dx == 2), tile_position=(0, 64))
                        nc.scalar.activation(hst[:, p0:p0 + 4, :, :], pp[:], AF.Relu,
                                             bias=bias_t["bi2c"][:, 0:1])
                        if g in (1, 3):
                            es1_site()
                    # h_lin2[(pixgroup, ch), s, q] (bf16, holds H_SCALE * h)
                    nc.sync.dma_start(out=h_lin2[0:64, 0:32:2, :],
                                      in_=hst[0:64, :, 0:5, :].rearrange("p k a b -> p k (a b)"))
                    nc.sync.dma_start(out=h_lin2[0:64, 1:32:2, :],
                                      in_=hst[64:128, :, 0:5, :].rearrange("p k a b -> p k (a b)"))
                    nc.sync.dma_start(out=h_lin2[64:128, 0:32:2, :],
                                      in_=hst[0:64, :, 5:10, :].rearrange("p k a b -> p k (a b)"))
                    nc.sync.dma_start(out=h_lin2[64:128, 1:32:2, :],
                                      in_=hst[64:128, :, 5:10, :].rearrange("p k a b -> p k (a b)"))
                    # fp8 pixel-major copy for the DoubleRow ihv matmuls
                    hq = pc.tile([128, 50, BC], fp8e4, tag="hq")
                    nc.vector.tensor_copy(hq[:], h_lin2[:].rearrange("p s q -> p q s"))
                    if "dbg_hlin" in debug:
                        nc.sync.dma_start(out=dbg("dbg_hlin", (128, BC, 50), bf16)[:], in_=h_lin2[:])

                    with tc.tile_pool(name="pssm", bufs=2, space="PSUM") as pssm:
                        # ---------- c2: dy-stacked 3x3 conv + pool, with the es
                        # tail stages interleaved (their serial fold chain hides
                        # behind c2's matmuls)
                        mark("c2")
                        for pair in range(16):
                            c2_pair(pair)
                            if pair >= 2 and pair % 2 == 0 and pair // 2 - 1 < 7:
                                estail_stage(pair // 2 - 1)
                        if "dbg_x2" in debug:
                            nc.sync.dma_start(out=dbg("dbg_x2", (128, BC, 10, 10), bf16)[:], in_=x2[:])
                        if "dbg_esT" in debug:
                            nc.sync.dma_start(out=dbg("dbg_esT", (128, 8, BC), bf16)[:], in_=esT[:])

                        # ---------- ihv: 25 fp8 DoubleRow matmuls (K=256 each)
                        mark("ihv")
                        p_ihv = pssm.tile([BC, 512], f32, tag="sm")
                        for q0, qn in [(0, 14), (14, 12), (26, 12), (38, 12)]:
                            if q0 not in wif_bufs:
                                wif_dma(q0, qn)
                            wifc = wif_bufs.pop(q0)
                            for k in range(qn // 2):
                                qg = q0 + 2 * k
                                nc.tensor.matmul(p_ihv[:], hq[:, qg:qg + 2, :],
                                                 wifc[:, 2 * k:2 * k + 2, :],
                                                 start=(qg == 0), stop=(qg == 48),
                                                 perf_mode=DR)
                        ihv_f = pc.tile([BC, 512], f32, tag="ihv_f")
                        nc.vector.tensor_tensor(ihv_f[:], p_ihv[:], bias_t["bifrep"][:], ALU.add)
                        ihvb = pc.tile([BC, 512], bf16, tag="ihvb")
                        nc.vector.tensor_scalar(ihvb[:], ihv_f[:], 0.0, None, ALU.max)
                        if "dbg_ihv" in debug:
                            nc.sync.dma_start(out=dbg("dbg_ihv", (BC, 512), bf16)[:], in_=ihvb[:])

                        for k in range(4):
                            pt = pssm.tile([128, BC], bf16, tag="sm")
                            nc.tensor.transpose(pt[:], ihvb[:, 128 * k:128 * (k + 1)], ident_t[0:32, :])
                            nc.vector.tensor_copy(catT[:, 4 + k, :], pt[:])
                        if "dbg_catT" in debug:
                            nc.sync.dma_start(out=dbg("dbg_catT", (128, 8, BC), bf16)[:], in_=catT[:])

                        # df emitted mid-c3 (after catT's ihv slots settle)
                        def df_stage():
                            pdf1 = pssm.tile([BC, 512], f32, tag="sm")
                            pdf2 = pssm.tile([BC, 16], f32, tag="sm")
                            for t in range(8):
                                nc.tensor.matmul(pdf1[:], catT[:, t, :], wdf_box["wdf"][:, t, 0:512],
                                                 start=(t == 0), stop=(t == 7))
                            for t in range(8):
                                nc.tensor.matmul(pdf2[:], catT[:, t, :], wdf_box["wdf"][:, t, 512:528],
                                                 start=(t == 0), stop=(t == 7))
                            wb_sb = pc.tile([BC, 528], f32, tag="wb_sb")
                            nc.vector.tensor_tensor(wb_sb[:, 0:512], pdf1[:],
                                                    bias_t["bdfrep"][:, 0:512], ALU.add)
                            nc.vector.tensor_tensor(wb_sb[:, 512:528], pdf2[:],
                                                    bias_t["bdfrep"][:, 512:528], ALU.add)
                            if "dbg_wb" in debug:
                                nc.sync.dma_start(out=dbg("dbg_wb", (BC, 528), f32)[:], in_=wb_sb[:])
                            return wb_sb

                        # ---------- c3
                        mark("c3")
                        wb_sb = None
                        for mt in range(2):
                            for sg in range(4):
                                pp = psp.tile([128, 8, 8, 8], f32, tag="mm")
                                first = True
                                for dy in range(3):
                                    for dx in range(3):
                                        nc.tensor.matmul(pp[:], w3_t[:, mt * 9 + dy * 3 + dx, :],
                                                         x2[:, sg * 8:(sg + 1) * 8, dy:dy + 8, dx:dx + 8],
                                                         start=first, stop=(dy == 2 and dx == 2))
                                        first = False
                                nc.scalar.activation(x3[:, mt, sg * 8:(sg + 1) * 8, :, :], pp[:],
                                                     AF.Relu, bias=bias_t["bc3"][:, mt:mt + 1])
                                if mt == 0 and sg == 2:
                                    wb_sb = df_stage()
                        if "dbg_x3" in debug:
                            nc.sync.dma_start(out=dbg("dbg_x3", (128, 2, BC, 8, 8), bf16)[:], in_=x3[:])

                        # ---------- c4 + pool
                        mark("c4")
                        for sg in range(4):
                            pp = psp.tile([64, 8, 6, 6], f32, tag="mm")
                            first = True
                            for kt in range(2):
                                for dy in range(3):
                                    for dx in range(3):
                                        nc.tensor.matmul(pp[:], w4_t[:, kt * 9 + dy * 3 + dx, :],
                                                         x3[:, kt, sg * 8:(sg + 1) * 8, dy:dy + 6, dx:dx + 6],
                                                         start=first, stop=(kt == 1 and dy == 2 and dx == 2))
                                        first = False
                            t4 = pc.tile([64, 8, 6, 6], bf16, tag="c4e")
                            nc.scalar.activation(t4[:], pp[:], AF.Relu, bias=bias_t["bc4"][:, 0:1])
                            h4 = pc.tile([64, 8, 6, 3], bf16, tag="c4h")
                            nc.vector.tensor_tensor(h4[:], t4[:, :, :, 0:6:2], t4[:, :, :, 1:6:2], ALU.max)
                            nc.vector.tensor_tensor(x4[:, sg * 8:(sg + 1) * 8, :, :],
                                                    h4[:, :, 0:6:2, :], h4[:, :, 1:6:2, :], ALU.max)

                        # ---------- c5 (batch-major out: samples on partitions)
                        mark("c5")
                        pp5 = pssm.tile([BC, 32], f32, tag="sm")
                        for q in range(9):
                            dy, dx = divmod(q, 3)
                            nc.tensor.matmul(pp5[:], x4[:, :, dy, dx], w5_t[:, q, :],
                                             start=(q == 0), stop=(q == 8))
                        xs_t = pc.tile([BC, 16, 2], f32, tag="xs")
                        xs_p = pc.tile([BC, 16, 2], f32, tag="xs_p")
                        nc.vector.tensor_tensor(xs_p[:], pp5[:].rearrange("p (a b) -> p a b", b=2),
                                                bias_t["b5rep"][:].rearrange("p (a b) -> p a b", b=2), ALU.add)
                        nc.vector.tensor_scalar(xs_t[:], xs_p[:], 0.0, None, ALU.max)
                        xg1 = pc.tile([BC, 16, 2], f32, tag="xg1")
                        nc.vector.tensor_copy(xg1[:], xs_t[:, :, ::-1])
                        if "dbg_xf" in debug:
                            nc.sync.dma_start(out=dbg("dbg_xf", (BC, 16, 2), f32)[:], in_=xs_t[:])

                        # ---------- dynamic 1x1 group conv tail (all DVE)
                        mark("tail")
                        wbv = wb_sb[:, 0:512].rearrange("p (o j) -> p o j", j=32)
                        tmp0 = pc.tile([BC, 16, 32], f32, tag="tmp0")
                        tmp1 = pc.tile([BC, 16, 32], f32, tag="tmp1")
                        xb0 = xs_t[:].rearrange("p a b -> p (a b)").unsqueeze(1).broadcast_to((BC, 16, 32))
                        xb1 = xg1[:].rearrange("p a b -> p (a b)").unsqueeze(1).broadcast_to((BC, 16, 32))
                        nc.vector.tensor_mul(tmp0[:], wbv, xb0)
                        nc.vector.tensor_mul(tmp1[:], wbv, xb1)
                        featr = pc.tile([BC, 16, 2], f32, tag="featr")
                        f0 = pc.tile([BC, 16], f32, tag="f0")
                        f1 = pc.tile([BC, 16], f32, tag="f1")
                        nc.vector.tensor_reduce(f0[:], tmp0[:], mybir.AxisListType.X, ALU.add)
                        nc.vector.tensor_reduce(f1[:], tmp1[:], mybir.AxisListType.X, ALU.add)
                        nc.vector.tensor_tensor(featr[:, :, 0], f0[:], wb_sb[:, 512:528], ALU.add)
                        nc.vector.tensor_tensor(featr[:, :, 1], f1[:], wb_sb[:, 512:528], ALU.add)
                        nc.vector.tensor_scalar(featr[:], featr[:], 0.0, None, ALU.max)
                        fb_ = featr[:].rearrange("p a b -> p (a b)").unsqueeze(1).broadcast_to((BC, 4, 32))
                        tmp2 = pc.tile([BC, 4, 32], f32, tag="tmp2")
                        nc.vector.tensor_mul(tmp2[:], bias_t["w2rep"][:], fb_)
                        o4_t = pc.tile([BC, 4], f32, tag="o4")
                        nc.vector.tensor_reduce(o4_t[:], tmp2[:], mybir.AxisListType.X, ALU.add)
                        outsb = pc.tile([BC, 4], f32, tag="outsb")
                        nc.vector.tensor_tensor(outsb[:], o4_t[:], bias_t["b2frep"][:], ALU.add)
                        nc.sync.dma_start(out=out_d[:], in_=outsb[:])
                        mark(None)

    nc.compile()
    return nc, dbg_handles


# ---------------------------------------------------------------- run
_CACHE = {}


def _get_module(debug=(), reps=1, sim=False):
    key = (tuple(sorted(debug)), reps, sim)
    if key not in _CACHE:
        _CACHE[key] = build(debug, reps, sim)
    return _CACHE[key]


def run(inputs, debug=()):
    nc, dbg_handles = _get_module(debug)
    in_maps = host_prep(inputs)
    res = run_bass_kernel_spmd(nc, in_maps, list(range(NCORES)))
    return res


def kernel(**inputs):
    res = run(inputs)
    out = np.concatenate([np.asarray(res.results[c]["out"], np.float32) for c in range(NCORES)], axis=0)
    return out.reshape(256, 2, 2)


# ---------------------------------------------------------------- timing
def make_runner(nc, in_maps):
    """Builds a reusable jitted executor for `nc` (mirrors
    bass2jax.run_bass_via_pjrt's multi-core path) with device-resident inputs.
    Returns a zero-arg callable that executes once and blocks."""
    import jax
    import numpy as _np
    from jax.sharding import Mesh, PartitionSpec
    from jax.experimental.shard_map import shard_map
    from concourse import bass2jax as b2j

    b2j.install_neuronx_cc_hook()
    n_cores = len(in_maps)
    partition_name = nc.partition_id_tensor.name if nc.partition_id_tensor else None
    in_names, out_names, out_avals, zero_outs = [], [], [], []
    for alloc in nc.m.functions[0].allocations:
        if not isinstance(alloc, mybir.MemoryLocationSet):
            continue
        name = alloc.memorylocations[0].name
        if alloc.kind == "ExternalInput":
            if name != partition_name:
                in_names.append(name)
        elif alloc.kind == "ExternalOutput":
            out_names.append(name)
            shape = tuple(alloc.tensor_shape)
            dtype = mybir.dt.np(alloc.dtype)
            out_avals.append(jax.core.ShapedArray(shape, dtype))
            zero_outs.append(_np.zeros(shape, dtype))
    n_params = len(in_names)
    n_outs = len(out_avals)
    all_in_names = list(in_names) + out_names
    if partition_name is not None:
        all_in_names.append(partition_name)

    def _body(*args):
        operands = list(args)
        if partition_name is not None:
            operands.append(b2j.partition_id_tensor())
        outs = b2j._bass_exec_p.bind(
            *operands,
            out_avals=tuple(out_avals),
            in_names=tuple(all_in_names),
            out_names=tuple(out_names),
            lowering_input_output_aliases=(),
            sim_require_finite=True,
            sim_require_nnan=True,
            nc=nc,
        )
        return tuple(outs)

    devices = jax.devices()[:n_cores]
    mesh = Mesh(_np.asarray(devices), ("core",))
    in_specs = (PartitionSpec("core"),) * (n_params + n_outs)
    out_specs = (PartitionSpec("core"),) * len(out_names)
    donate = tuple(range(n_params, n_params + n_outs))
    sharded = jax.jit(
        shard_map(_body, mesh=mesh, in_specs=in_specs, out_specs=out_specs,
                  check_rep=False),
        donate_argnums=donate, keep_unused=True)
    concat_in = [
        _np.concatenate([_np.asarray(in_maps[c][nm]) for c in range(n_cores)], axis=0)
        for nm in in_names
    ]
    from jax.sharding import NamedSharding
    shard = NamedSharding(mesh, PartitionSpec("core"))
    in_dev = [jax.device_put(a, shard) for a in concat_in]
    zshapes = [((n_cores * z.shape[0],) + z.shape[1:], z.dtype) for z in zero_outs]

    def call():
        zs = [jax.device_put(_np.zeros(s, d), shard) for s, d in zshapes]
        outs = sharded(*in_dev, *zs)
        jax.block_until_ready(outs)
        return outs

    return call


def time_kernel_reps(inputs, iters=8, reps=4):
    """Differential in-program repetition timing with PAIRED alternation:
    builds reps=1 and reps=N modules, alternates r1/rN calls so slow drift in
    the shared box cancels, and reports the median paired difference /(N-1)."""
    import time
    in_maps = host_prep(inputs)
    nc1, _ = _get_module((), 1)
    call1 = make_runner(nc1, in_maps)
    ncN, _ = _get_module((), reps)
    callN = make_runner(ncN, in_maps)
    call1(); callN(); call1(); callN()
    diffs, t1s, tNs = [], [], []
    for _ in range(iters):
        t0 = time.perf_counter()
        call1()
        t1 = time.perf_counter()
        callN()
        t2 = time.perf_counter()
        t1s.append(t1 - t0)
        tNs.append(t2 - t1)
        diffs.append((t2 - t1) - (t1 - t0))
    import numpy as _np
    med = _np.median(diffs)
    return med / (reps - 1) * 1e9, _np.median(t1s) * 1e9, _np.median(tNs) * 1e9


def time_kernel(inputs, iters=10):
    """Returns (best_ns, floor_ns): wall time of one kernel execution and of a
    null kernel through the same dispatch path."""
    import time
    nc, _ = _get_module(())
    in_maps = host_prep(inputs)
    call = make_runner(nc, in_maps)
    call()
    ts = []
    for _ in range(iters):
        t0 = time.perf_counter()
        call()
        ts.append(time.perf_counter() - t0)
    best = min(ts)

    # null kernel floor
    key = "_null"
    if key not in _CACHE:
        ncn = bacc.Bacc("TRN2", target_bir_lowering=False, debug=False, num_devices=NCORES)
        xi = ncn.dram_tensor("x", [128, 4], f32, kind="ExternalInput")
        yo = ncn.dram_tensor("y", [128, 4], f32, kind="ExternalOutput")
        with tile.TileContext(ncn) as tcn:
            with tcn.tile_pool(name="p", bufs=1) as pool:
                t = pool.tile([128, 4], f32)
                ncn.sync.dma_start(out=t[:], in_=xi[:])
                ncn.sync.dma_start(out=yo[:], in_=t[:])
        ncn.compile()
        _CACHE[key] = ncn
    ncn = _CACHE[key]
    calln = make_runner(ncn, [{"x": np.zeros((128, 4), np.float32)}] * NCORES)
    calln()
    tn = []
    for _ in range(iters):
        t0 = time.perf_counter()
        calln()
        tn.append(time.perf_counter() - t0)
    floor = min(tn)
    return best * 1e9, floor * 1e9
